# revision 1
# baseline (speedup 1.0000x reference)
"""DiversityAttention on 8 TRN2 NeuronCores (Bass/Tile).

Sharding: data-parallel over batch (B=2) x tensor-parallel over heads
(16 heads -> 4 groups of 4). core = (b, g), b = core // 4, g = core % 4.
Each core computes full attention for its 4 heads over its batch and a
partial out-projection [S, HIDDEN]; the host sums the 4 partials per
batch and adds bo.

Device-side formulation (keys-on-partitions / "S^T" orientation, so no
large transposes are ever needed):
  qT = (Wq/8 @ x^T + bq/8)   [64h, S]   (1/sqrt(dh) folded into Wq on host)
  kT = (Wk   @ x^T + bk)     [64h, S]
  vT = (Wv   @ x^T + bv) then PE-transposed to V [S, 64h] (+ ones col)
  xh = sqrt(gamma) * x^T / max(||x||, eps)  (in-place columns scale of xT)
  per (ktile, qblock): sim_psum[k,q] = xh^T xh ; per head:
     scores_psum[k,q] = kT^T qT ; P = exp(scores - sim) (DVE sub, ACT exp)
  ctx^T[d,q] (+sums row) = sum_k [V|1]^T P  accumulated in PSUM
  ctx normalized by reciprocal(sums) (PE broadcast outer product)
  out[q,o] partial = ctxT^T @ WoT  -> DMA to DRAM

All matmuls run as float32r (full PE rate at N>=256, ~fp32 precision).
"""

import math
import os
import sys

import numpy as np

for _p in ("/opt/trn_rl_repo",):
    if _p not in sys.path and os.path.isdir(_p):
        sys.path.insert(0, _p)

os.environ.setdefault("MYCRO_LOCAL_CACHE", "1")

import concourse.bass as bass
import concourse.tile as tile
from concourse import bacc, mybir
from concourse.bass_utils import run_bass_kernel_spmd
from concourse.masks import make_identity


def _install_ntff_hook():
    """Provide antenv.axon_hooks (NTFF profiling registry) if the image
    lacks it, mirroring trn_agent_boot's ctypes hook. No-op on failure."""
    try:
        import antenv.axon_hooks  # noqa: F401
        return
    except ImportError:
        pass
    try:
        import contextlib
        import ctypes
        import types

        so_path = "/opt/axon/libaxon_pjrt.so"
        if not os.path.exists(so_path):
            return
        lib = ctypes.CDLL(so_path)
        if not hasattr(lib, "axon_start_nrt_profile"):
            return
        lib.axon_start_nrt_profile.argtypes = [
            ctypes.POINTER(ctypes.c_int64), ctypes.c_size_t]
        lib.axon_start_nrt_profile.restype = ctypes.c_int64
        lib.axon_stop_nrt_profile.argtypes = [ctypes.c_char_p]
        lib.axon_stop_nrt_profile.restype = ctypes.c_int64

        @contextlib.contextmanager
        def _hook(output_dir, device_ids):
            import jax
            jax.devices()
            if device_ids:
                ids = (ctypes.c_int64 * len(device_ids))(*device_ids)
                rc = lib.axon_start_nrt_profile(ids, len(device_ids))
            else:
                rc = lib.axon_start_nrt_profile(None, 0)
            if rc != 0:
                raise RuntimeError(f"axon_start_nrt_profile rc={rc}")
            try:
                yield
            finally:
                n = lib.axon_stop_nrt_profile(str(output_dir).encode())
                print(f"ntff profile: {n} file(s) -> {output_dir}",
                      file=sys.stderr)

        mod = types.ModuleType("antenv.axon_hooks")
        _state = {"hook": _hook}
        mod.set_axon_ntff_profile_hook = lambda h: _state.__setitem__("hook", h)
        mod.get_axon_ntff_profile_hook = lambda: _state["hook"]
        sys.modules["antenv.axon_hooks"] = mod
        import antenv
        antenv.axon_hooks = mod
    except Exception:
        pass


_install_ntff_hook()

F32 = mybir.dt.float32
F32R = mybir.dt.float32r
ALU = mybir.AluOpType
ACT_EXP = mybir.ActivationFunctionType.Exp
ACT_COPY = mybir.ActivationFunctionType.Copy

# Problem constants (hardcoded per contract).
HIDDEN = 1024
HEADS = 16
HEAD_DIM = 64
GAMMA = 0.5
B, S = 2, 2048
N_CORES = 8
GROUPS = N_CORES // B  # head groups per batch
HPC = HEADS // GROUPS  # heads per core
LAG = 2  # kt software-pipeline lag between exp and ctx matmul


def _r(ap):
    return ap.bitcast(F32R)


def emit_kernel(tc, aps, *, S_, C_, HPC_, QB):
    """Emit the per-core kernel. aps: dict of dram APs."""
    nc = tc.nc
    CT = C_ // 128          # contraction tiles over hidden
    PAIRS = HPC_ // 2       # head pairs (128-channel chunks)
    NKT = S_ // 128         # key tiles
    NQB = S_ // QB          # query blocks
    PB = min(512, S_)       # projection free-block width
    NPB = S_ // PB
    OB_W = min(512, C_)     # out-projection free-block width
    NOB = C_ // OB_W

    xT_d = aps["xT"]; scale_d = aps["scale"]
    wq_d = aps["wq"]; wk_d = aps["wk"]; wv_d = aps["wv"]; wo_d = aps["wo"]
    bq_d = aps["bq"]; bk_d = aps["bk"]; bv_d = aps["bv"]
    out_d = aps["out"]
    mask_d = aps.get("maskadd")

    from contextlib import ExitStack
    stack = ExitStack()
    consts = stack.enter_context(tc.tile_pool(name="consts", bufs=1))
    xpool = stack.enter_context(tc.tile_pool(name="xpool", bufs=1))
    projpool = stack.enter_context(tc.tile_pool(name="projpool", bufs=1))

    # --- constants ---
    identity = consts.tile([128, 128], F32)
    make_identity(nc, identity)

    wo_sb = consts.tile([128, PAIRS, C_], F32R)

    # x^T loaded in chunks and rounded to fp32r by the scalar engine
    xTr = xpool.tile([128, CT, S_], F32R)

    # projections (fp32r for q/k; plain f32 for v which feeds the transpose)
    qT_sb = projpool.tile([128, PAIRS, S_], F32R)
    kT_sb = projpool.tile([128, PAIRS, S_], F32R)
    v2_sb = projpool.tile([128, HPC_, NKT, HEAD_DIM + 1], F32R)

    with tc.tile_pool(name="xstage", bufs=2) as xstage, \
         tc.tile_pool(name="wstage", bufs=1) as wstage, \
         tc.tile_pool(name="wpool", bufs=1) as wpool, \
         tc.tile_pool(name="vstage", bufs=1) as vstage, \
         tc.tile_pool(name="ph1psum", bufs=2, space="PSUM") as prj_ps, \
         tc.tile_pool(name="tppsum", bufs=4, space="PSUM") as tp_ps:
        # load + round x^T
        for c in range(CT):
            xs = xstage.tile([128, S_], F32, tag="xs")
            nc.sync.dma_start(out=xs, in_=xT_d[c * 128:(c + 1) * 128, :])
            nc.scalar.activation(out=xTr[:, c, :], in_=xs, func=ACT_COPY)
        # load + round weights (DVE)
        wq_sb = wpool.tile([128, CT, D2_of(HPC_)], F32R)
        wk_sb = wpool.tile([128, CT, D2_of(HPC_)], F32R)
        wv_sb = wpool.tile([128, CT, D2_of(HPC_)], F32R)
        for w_sb, w_d in ((wq_sb, wq_d), (wk_sb, wk_d), (wv_sb, wv_d)):
            ws = wstage.tile([128, CT, D2_of(HPC_)], F32, tag="ws")
            nc.sync.dma_start(out=ws, in_=w_d.rearrange("(t p) m -> p t m", p=128))
            nc.vector.tensor_copy(w_sb, ws)
        wos = wstage.tile([128, PAIRS, C_], F32, tag="ws")
        nc.sync.dma_start(out=wos, in_=wo_d.rearrange("(j p) o -> p j o", p=128))
        nc.vector.tensor_copy(wo_sb, wos)
        bq_sb = wpool.tile([128, PAIRS, 1], F32)
        bk_sb = wpool.tile([128, PAIRS, 1], F32)
        bv_sb = wpool.tile([128, PAIRS, 1], F32)
        for b_sb, b_d in ((bq_sb, bq_d), (bk_sb, bk_d), (bv_sb, bv_d)):
            nc.sync.dma_start(
                out=b_sb, in_=b_d.rearrange("(j p) one -> p j one", p=128))

        vT_sb = vstage.tile([128, PAIRS, S_], F32)
        for w_sb, b_sb, dest in (
            (wq_sb, bq_sb, qT_sb),
            (wk_sb, bk_sb, kT_sb),
            (wv_sb, bv_sb, vT_sb),
        ):
            for nb in range(NPB):
                pss = [prj_ps.tile([128, PB], F32, tag=f"prj{j}",
                                   name=f"prj_{dest.tensor.name}_{nb}_{j}")
                       for j in range(PAIRS)]
                for c in range(CT):
                    for j in range(PAIRS):
                        nc.tensor.matmul(
                            pss[j],
                            w_sb[:, c, j * 128:(j + 1) * 128],
                            xTr[:, c, nb * PB:(nb + 1) * PB],
                            start=(c == 0),
                            stop=(c == CT - 1),
                        )
                for j in range(PAIRS):
                    nc.vector.tensor_scalar_add(
                        dest[:, j, nb * PB:(nb + 1) * PB], pss[j], b_sb[:, j, :]
                    )

        # V: PE-transpose vT (f32) -> [keys, d] layout, 2 heads per tile
        for j in range(PAIRS):
            for t in range(NKT):
                tp = tp_ps.tile([128, 128], F32, tag="tp")
                nc.tensor.transpose(tp, vT_sb[:, j, t * 128:(t + 1) * 128], identity)
                nc.scalar.activation(
                    out=v2_sb[:, 2 * j:2 * j + 2, t, 0:HEAD_DIM],
                    in_=tp.rearrange("p (h d) -> p h d", h=2),
                    func=ACT_COPY,
                )
        onescol = wstage.tile([128, HPC_, NKT, 1], F32)
        nc.vector.memset(onescol, 1.0)
        nc.vector.tensor_copy(v2_sb[:, :, :, HEAD_DIM:HEAD_DIM + 1], onescol)

    # xT -> xh in place: multiply columns by sqrt(gamma)/||x_row||
    ctxT2_sb = projpool.tile([128, PAIRS, S_], F32R)
    with tc.tile_pool(name="bcpool", bufs=1) as bcpool:
        bcast_sb = bcpool.tile([128, S_], F32)
        nc.sync.dma_start(out=bcast_sb, in_=scale_d.to_broadcast([128, S_]))
        for c in range(CT):
            nc.vector.tensor_mul(xTr[:, c, :], xTr[:, c, :], bcast_sb)

    # --- main loop (phase 2) ---
    ptpool = stack.enter_context(tc.tile_pool(name="ptpool", bufs=7))
    spool = stack.enter_context(tc.tile_pool(name="spool", bufs=2))
    simsb = stack.enter_context(tc.tile_pool(name="simsb", bufs=2))
    smallpool = stack.enter_context(tc.tile_pool(name="smallpool", bufs=2))
    mpool = (stack.enter_context(tc.tile_pool(name="mpool", bufs=2))
             if mask_d is not None else None)

    with tc.tile_pool(name="simpsum", bufs=2, space="PSUM") as simp, \
         tc.tile_pool(name="scpsum", bufs=1, space="PSUM") as scp, \
         tc.tile_pool(name="ctxpsum", bufs=1, space="PSUM") as ctxp:

        def emit_ctx(ctx_ps, kt, pt_pairs):
            for j in range(PAIRS):
                for hi in range(2):
                    nc.tensor.matmul(
                        ctx_ps[2 * j + hi],
                        v2_sb[:, 2 * j + hi, kt, :],
                        pt_pairs[j][:, hi, :],
                        start=(kt == 0),
                        stop=(kt == NKT - 1),
                        skip_group_check=True,
                    )

        def emit_division_head(qb, ctx_ps, h):
            j, hi = divmod(h, 2)
            r0 = smallpool.tile([1, QB], F32, tag=f"r0{h % 2}",
                                name=f"r0_{qb}_{h}")
            nc.vector.reciprocal(
                r0, ctx_ps[h][HEAD_DIM:HEAD_DIM + 1, :])
            rb = smallpool.tile([HEAD_DIM, QB], F32, tag="rb")
            nc.gpsimd.partition_broadcast(rb, r0, channels=HEAD_DIM)
            nc.vector.tensor_mul(
                ctxT2_sb[hi * 64:hi * 64 + 64, j, qb * QB:(qb + 1) * QB],
                ctx_ps[h][0:HEAD_DIM, :],
                rb,
            )

        def emit_division(qb, ctx_ps):
            for h in range(HPC_):
                emit_division_head(qb, ctx_ps, h)

        prev_div = None
        for qb in range(NQB):
            ctx_ps = [ctxp.tile([HEAD_DIM + 1, QB], F32, tag=f"ctx{h}",
                                name=f"ctx_{qb}_{h}")
                      for h in range(HPC_)]
            pending = []
            for kt in range(NKT):
                if prev_div is not None and kt >= 2 and (kt - 2) % 3 == 0:
                    h = (kt - 2) // 3
                    if h < HPC_:
                        emit_division_head(prev_div[0], prev_div[1], h)
                        if h == HPC_ - 1:
                            prev_div = None
                sp = simp.tile([128, QB], F32, tag="sim")
                for c in range(CT):
                    nc.tensor.matmul(
                        sp,
                        xTr[:, c, kt * 128:(kt + 1) * 128],
                        xTr[:, c, qb * QB:(qb + 1) * QB],
                        start=(c == 0),
                        stop=(c == CT - 1),
                    )
                sim_t = simsb.tile([128, QB], F32, tag="simsb")
                nc.scalar.activation(out=sim_t, in_=sp, func=ACT_COPY)
                if mask_d is not None:
                    m_sb = mpool.tile([128, QB], F32, tag="msk")
                    nc.sync.dma_start(
                        out=m_sb,
                        in_=mask_d[kt * 128:(kt + 1) * 128, qb * QB:(qb + 1) * QB],
                    )
                    nc.vector.tensor_sub(sim_t, sim_t, m_sb)
                pt_pairs = []
                for j in range(PAIRS):
                    sc_t = scp.tile([128, 2, QB], F32, tag="scp")
                    for hi in range(2):
                        pr = slice(hi * 64, hi * 64 + 64)
                        nc.tensor.matmul(
                            sc_t[:, hi, :],
                            kT_sb[pr, j, kt * 128:(kt + 1) * 128],
                            qT_sb[pr, j, qb * QB:(qb + 1) * QB],
                            start=True,
                            stop=True,
                        )
                    s_t = spool.tile([128, 2, QB], F32, tag="s")
                    nc.vector.tensor_sub(
                        s_t, sc_t,
                        sim_t.unsqueeze(1).to_broadcast([128, 2, QB]))
                    pt = ptpool.tile([128, 2, QB], F32R, tag="pt")
                    nc.scalar.activation(out=pt, in_=s_t, func=ACT_EXP)
                    pt_pairs.append(pt)
                pending.append((kt, pt_pairs))
                if len(pending) > LAG:
                    k0, p0 = pending.pop(0)
                    emit_ctx(ctx_ps, k0, p0)
            for k0, p0 in pending:
                emit_ctx(ctx_ps, k0, p0)
            if prev_div is not None:
                done = max(0, (NKT - 1 - 2) // 3 + 1) if NKT > 2 else 0
                for h in range(min(done, HPC_), HPC_):
                    emit_division_head(prev_div[0], prev_div[1], h)
                prev_div = None
            prev_div = (qb, ctx_ps)
        emit_division(*prev_div)

    # --- out-projection (phase 3) ---
    with tc.tile_pool(name="outpsum", bufs=4, space="PSUM") as outp, \
         tc.tile_pool(name="outstg", bufs=4) as outstg:
        for qt in range(S_ // 128):
            for ob in range(NOB):
                op = outp.tile([128, OB_W], F32, tag="op")
                for j in range(PAIRS):
                    nc.tensor.matmul(
                        op,
                        ctxT2_sb[:, j, qt * 128:(qt + 1) * 128],
                        wo_sb[:, j, ob * OB_W:(ob + 1) * OB_W],
                        start=(j == 0),
                        stop=(j == PAIRS - 1),
                    )
                ostg = outstg.tile([128, OB_W], F32, tag="ostg")
                if (qt + ob) % 2 == 0:
                    nc.scalar.activation(out=ostg, in_=op, func=ACT_COPY)
                else:
                    nc.vector.tensor_copy(ostg, op)
                nc.sync.dma_start(
                    out=out_d[qt * 128:(qt + 1) * 128, ob * OB_W:(ob + 1) * OB_W],
                    in_=ostg,
                )

    stack.close()


def D2_of(hpc):
    return hpc * HEAD_DIM


def build_nc(*, S_=S, C_=HIDDEN, HPC_=HPC, QB=512, with_mask=False,
             enable_asserts=False):
    nc = bacc.Bacc(
        "TRN2", target_bir_lowering=False, debug=False,
        enable_asserts=enable_asserts,
    )
    D2 = HPC_ * HEAD_DIM
    aps = {}
    aps["xT"] = nc.dram_tensor("xT", [C_, S_], F32, kind="ExternalInput").ap()
    aps["scale"] = nc.dram_tensor("scale", [1, S_], F32, kind="ExternalInput").ap()
    for n in ("wq", "wk", "wv"):
        aps[n] = nc.dram_tensor(n, [C_, D2], F32, kind="ExternalInput").ap()
    aps["wo"] = nc.dram_tensor("wo", [D2, C_], F32, kind="ExternalInput").ap()
    for n in ("bq", "bk", "bv"):
        aps[n] = nc.dram_tensor(n, [D2, 1], F32, kind="ExternalInput").ap()
    if with_mask:
        aps["maskadd"] = nc.dram_tensor(
            "maskadd", [S_, S_], F32, kind="ExternalInput").ap()
    aps["out"] = nc.dram_tensor("out", [S_, C_], F32, kind="ExternalOutput").ap()

    with tile.TileContext(nc) as tc:
        emit_kernel(tc, aps, S_=S_, C_=C_, HPC_=HPC_, QB=QB)
    nc.compile()
    return nc


def host_prepare(x, attn_mask, Wq, bq, Wk, bk, Wv, bv, Wo, bo, *,
                 S_=S, C_=HIDDEN, HPC_=HPC, n_cores=N_CORES):
    """Build the per-core input maps. Returns (in_maps, with_mask)."""
    x = np.asarray(x, np.float32)
    B_ = x.shape[0]
    groups = n_cores // B_
    Wq = np.asarray(Wq, np.float32); Wk = np.asarray(Wk, np.float32)
    Wv = np.asarray(Wv, np.float32); Wo = np.asarray(Wo, np.float32)
    bq = np.asarray(bq, np.float32); bk = np.asarray(bk, np.float32)
    bv = np.asarray(bv, np.float32)

    inv_sqrt_d = 1.0 / math.sqrt(HEAD_DIM)
    WqT = np.ascontiguousarray((Wq * inv_sqrt_d).T)  # [C, C] in->out
    WkT = np.ascontiguousarray(Wk.T)
    WvT = np.ascontiguousarray(Wv.T)
    WoT = np.ascontiguousarray(Wo.T)                 # [C(c), C(o)]
    bq = bq * inv_sqrt_d

    mask = np.asarray(attn_mask)
    with_mask = bool(mask.any())
    maskadd = None
    if with_mask:
        # reference: where(mask, -inf); use a large negative additive bias
        maskadd = np.where(mask, np.float32(-1e30), np.float32(0.0)).astype(np.float32)
        # device layout: maskadd[k, q] added to scores^T
        maskadd = np.ascontiguousarray(maskadd.T)  # [k, q] = mask[q, k].T

    in_maps = []
    for core in range(n_cores):
        b, g = divmod(core, groups)
        xb = x[b]                                   # [S, C]
        xT = np.ascontiguousarray(xb.T)             # [C, S]
        norms = np.linalg.norm(xb, axis=1)          # [S]
        scale = (math.sqrt(GAMMA) / np.maximum(norms, 1e-12)).astype(np.float32)
        ch = slice(g * HPC_ * HEAD_DIM, (g + 1) * HPC_ * HEAD_DIM)
        m = {
            "xT": xT,
            "scale": scale.reshape(1, S_),
            "wq": np.ascontiguousarray(WqT[:, ch]),
            "wk": np.ascontiguousarray(WkT[:, ch]),
            "wv": np.ascontiguousarray(WvT[:, ch]),
            "wo": np.ascontiguousarray(WoT[ch, :]),
            "bq": np.ascontiguousarray(bq[ch]).reshape(-1, 1),
            "bk": np.ascontiguousarray(bk[ch]).reshape(-1, 1),
            "bv": np.ascontiguousarray(bv[ch]).reshape(-1, 1),
        }
        if with_mask:
            m["maskadd"] = maskadd
        in_maps.append(m)
    return in_maps, with_mask


_NC_CACHE = {}


def _get_nc(with_mask):
    key = with_mask
    if key not in _NC_CACHE:
        _NC_CACHE[key] = build_nc(with_mask=with_mask)
    return _NC_CACHE[key]


LAST_RESULTS = None


def kernel(**inputs):
    global LAST_RESULTS
    in_maps, with_mask = host_prepare(
        inputs["x"], inputs["attn_mask"],
        inputs["Wq"], inputs["bq"], inputs["Wk"], inputs["bk"],
        inputs["Wv"], inputs["bv"], inputs["Wo"], inputs["bo"],
    )
    nc = _get_nc(with_mask)
    res = run_bass_kernel_spmd(nc, in_maps, core_ids=list(range(N_CORES)))
    LAST_RESULTS = res
    bo = np.asarray(inputs["bo"], np.float32)
    out = np.zeros((B, S, HIDDEN), np.float32)
    groups = N_CORES // B
    for core in range(N_CORES):
        b = core // groups
        out[b] += res.results[core]["out"]
    out += bo[None, None, :]
    return out



# revision 7
# speedup vs baseline: 1.4673x; 1.4673x over previous
"""DiversityAttention on 8 TRN2 NeuronCores (Bass/Tile), v2.

Sharding: data-parallel over batch (B=2) x tensor-parallel over heads
(16 heads -> 4 groups of 4). core = (b, g), b = core // 4, g = core % 4.
Each core computes full attention for its 4 heads over its batch and a
partial out-projection [S, HIDDEN]; the host sums the 4 partials per
batch and adds bo.

Key structure (keys-on-partitions orientation, no transposes):
  qT = (Wq/8 @ x^T + bq/8)  [64h, S] bf16    (1/sqrt(dh) folded on host)
  kT = (Wk   @ x^T + bk)    [64h, S] bf16
  V  = x @ WvT + bv directly in [keys, dh] layout (x-tile stationary),
       bf16, with a ones column for softmax sums
  xh8 = fp8e4(x^T * 64*sqrt(gamma)/||x||)  (DoubleRow-interleaved layout)
  per (qb, kt):
     sim_psum = xh8^T xh8 (fp8 DoubleRow, 2x rate) = 4096*gamma*sim
     E = exp(-sim_psum/4096) (ACT, scale folded)          [128, QB] bf16
     scores_psum = kT^T qT per head pair (bf16, row-packed 64-contraction)
     es = exp(scores_psum) (ACT reads PSUM directly)      bf16
     pt = es * E (DVE bf16 2x mode)
     ctx_psum[65, QB] += [V|1]^T pt  (accumulated over kt, per head)
  at qb end: recip_approx_fast(sums) -> gpsimd broadcast -> DVE mul
     gives ctxT2 [dh, q] bf16; out-projection of qb is emitted inside
     the next qb's loop (PE never idles at block boundaries).
"""

import math
import os
import sys

import numpy as np

for _p in ("/opt/trn_rl_repo",):
    if _p not in sys.path and os.path.isdir(_p):
        sys.path.insert(0, _p)

os.environ.setdefault("MYCRO_LOCAL_CACHE", "1")

import concourse.bass as bass
import concourse.tile as tile
from concourse import bacc, mybir
from concourse.bass_utils import run_bass_kernel_spmd


def _install_ntff_hook():
    """Provide antenv.axon_hooks (NTFF profiling registry) if the image
    lacks it, mirroring trn_agent_boot's ctypes hook. No-op on failure."""
    try:
        import antenv.axon_hooks  # noqa: F401
        return
    except ImportError:
        pass
    try:
        import contextlib
        import ctypes
        import types

        so_path = "/opt/axon/libaxon_pjrt.so"
        if not os.path.exists(so_path):
            return
        lib = ctypes.CDLL(so_path)
        if not hasattr(lib, "axon_start_nrt_profile"):
            return
        lib.axon_start_nrt_profile.argtypes = [
            ctypes.POINTER(ctypes.c_int64), ctypes.c_size_t]
        lib.axon_start_nrt_profile.restype = ctypes.c_int64
        lib.axon_stop_nrt_profile.argtypes = [ctypes.c_char_p]
        lib.axon_stop_nrt_profile.restype = ctypes.c_int64

        @contextlib.contextmanager
        def _hook(output_dir, device_ids):
            import jax
            jax.devices()
            if device_ids:
                ids = (ctypes.c_int64 * len(device_ids))(*device_ids)
                rc = lib.axon_start_nrt_profile(ids, len(device_ids))
            else:
                rc = lib.axon_start_nrt_profile(None, 0)
            if rc != 0:
                raise RuntimeError(f"axon_start_nrt_profile rc={rc}")
            try:
                yield
            finally:
                n = lib.axon_stop_nrt_profile(str(output_dir).encode())
                print(f"ntff profile: {n} file(s) -> {output_dir}",
                      file=sys.stderr)

        mod = types.ModuleType("antenv.axon_hooks")
        _state = {"hook": _hook}
        mod.set_axon_ntff_profile_hook = lambda h: _state.__setitem__("hook", h)
        mod.get_axon_ntff_profile_hook = lambda: _state["hook"]
        sys.modules["antenv.axon_hooks"] = mod
        import antenv
        antenv.axon_hooks = mod
    except Exception:
        pass


_install_ntff_hook()

F32 = mybir.dt.float32
F32R = mybir.dt.float32r
BF16 = mybir.dt.bfloat16
FP8 = mybir.dt.float8e4
ALU = mybir.AluOpType
ACT_EXP = mybir.ActivationFunctionType.Exp
ACT_COPY = mybir.ActivationFunctionType.Copy
DR = mybir.MatmulPerfMode.DoubleRow

# Problem constants (hardcoded per contract).
HIDDEN = 1024
HEADS = 16
HEAD_DIM = 64
GAMMA = 0.5
B, S = 2, 2048
N_CORES = 8
GROUPS = N_CORES // B  # head groups per batch
HPC = HEADS // GROUPS  # heads per core
PAIRS = HPC // 2
LAG = 3                # kt software-pipeline lag between pt and ctx matmul
XH_PRESCALE = 64.0     # fp8 prescale; sim psum = PRESCALE^2 * gamma * sim


def emit_kernel(tc, aps, *, S_, C_, QB):
    nc = tc.nc
    CT = C_ // 128          # contraction tiles over hidden
    CC = CT // 2            # fp8 DoubleRow chunks (256 rows each)
    NKT = S_ // 128         # key tiles
    NQB = S_ // QB          # query blocks
    PB = 512                # projection free-block width
    NPB = S_ // PB
    OB_W = 512              # out-projection free-block width
    NOB = C_ // OB_W
    D2 = HPC * HEAD_DIM     # per-core projected channels

    xT_d = aps["xT"]; scale_d = aps["scale"]
    wq_d = aps["wq"]; wk_d = aps["wk"]; wv_d = aps["wv"]; wo_d = aps["wo"]
    bq_d = aps["bq"]; bk_d = aps["bk"]; bv_d = aps["bv"]
    out_d = aps["out"]
    mask_d = aps.get("maskmul")

    from contextlib import ExitStack
    stack = ExitStack()

    # --- persistent SBUF tensors ---
    proj = stack.enter_context(tc.tile_pool(name="proj", bufs=1))
    qT_sb = proj.tile([128, PAIRS, S_], BF16)      # head pairs on 64-halves
    kT_sb = proj.tile([128, PAIRS, S_], BF16)
    v2_sb = proj.tile([128, NKT, HPC, HEAD_DIM + 1], BF16)
    xh8_sb = proj.tile([128, CC, 2, S_], FP8)      # DoubleRow interleaved
    wo_sb = proj.tile([128, PAIRS, C_], BF16)
    ctxT2_sb = proj.tile([128, PAIRS, S_], BF16)

    # ---------------- phase 1: load + projections ----------------
    with tc.tile_pool(name="xsp", bufs=1) as xsp, \
         tc.tile_pool(name="wsp", bufs=1) as wsp, \
         tc.tile_pool(name="prjps", bufs=2, space="PSUM") as prjps, \
         tc.tile_pool(name="vps", bufs=2, space="PSUM") as vps:

        # weights first (small), then x chunks; fine-grained tiles for deps
        w_sb = {}
        for wname, w_d in (("wq", wq_d), ("wk", wk_d), ("wv", wv_d)):
            for c in range(CT):
                wt = wsp.tile([128, D2], F32R, tag=f"{wname}{c}",
                              name=f"{wname}_{c}")
                nc.sync.dma_start(out=wt, in_=w_d[c * 128:(c + 1) * 128, :])
                w_sb[(wname, c)] = wt
        wos = wsp.tile([128, PAIRS, C_], F32, tag="wos")
        nc.sync.dma_start(out=wos, in_=wo_d.rearrange("(j p) o -> p j o", p=128))
        nc.vector.tensor_copy(wo_sb, wos)

        b_sb = {}
        for bname, b_d in (("bq", bq_d), ("bk", bk_d), ("bv", bv_d)):
            bt = wsp.tile([128, PAIRS, 1], F32, tag=f"b{bname}")
            nc.sync.dma_start(
                out=bt, in_=b_d.rearrange("(j p) one -> p j one", p=128))
            b_sb[bname] = bt

        # normalization scale row -> broadcast to all partitions (gpsimd)
        scr = wsp.tile([1, S_], F32R, tag="scr")
        nc.sync.dma_start(out=scr, in_=scale_d)
        scb = wsp.tile([128, S_], F32R, tag="scb")
        nc.gpsimd.partition_broadcast(scb, scr, channels=128)

        # bv broadcast row for the V bias add ([1, D2] varies along free dim)
        bvr = wsp.tile([1, D2], F32, tag="bvr")
        nc.sync.dma_start(out=bvr, in_=bv_d.rearrange("d one -> one d"))
        bvb = wsp.tile([128, D2], F32, tag="bvb")
        nc.gpsimd.partition_broadcast(bvb, bvr, channels=128)

        xs = []
        for c in range(CT):
            xt = xsp.tile([128, S_], F32R, tag=f"xs{c}", name=f"xs_{c}")
            nc.sync.dma_start(out=xt, in_=xT_d[c * 128:(c + 1) * 128, :])
            xs.append(xt)

        # q/k projections: W tiles stationary, x moving; accumulate over c
        for wname, bname, dest in (("wq", "bq", qT_sb), ("wk", "bk", kT_sb)):
            for nb in range(NPB):
                pss = [prjps.tile([128, PB], F32, tag=f"prj{j}",
                                  name=f"prj_{wname}_{nb}_{j}")
                       for j in range(PAIRS)]
                for c in range(CT):
                    xr = xs[c]
                    wr = w_sb[(wname, c)]
                    for j in range(PAIRS):
                        nc.tensor.matmul(
                            pss[j],
                            wr[:, j * 128:(j + 1) * 128],
                            xr[:, nb * PB:(nb + 1) * PB],
                            start=(c == 0),
                            stop=(c == CT - 1),
                        )
                for j in range(PAIRS):
                    nc.vector.tensor_scalar_add(
                        dest[:, j, nb * PB:(nb + 1) * PB], pss[j],
                        b_sb[bname][:, j, :])

        # V directly in [keys, dh] layout: x tile stationary, WvT moving
        for st in range(NKT):
            vp = vps.tile([128, D2], F32, tag="vp", name=f"vp_{st}")
            for c in range(CT):
                xr = xs[c]
                wr = w_sb[("wv", c)]
                nc.tensor.matmul(
                    vp,
                    xr[:, st * 128:(st + 1) * 128],
                    wr,
                    start=(c == 0),
                    stop=(c == CT - 1),
                )
            # v2[:, st, h, 0:64] = vp + bv  (strided write, bf16)
            nc.vector.tensor_add(
                v2_sb[:, st, :, 0:HEAD_DIM],
                vp.rearrange("p (h d) -> p h d", h=HPC),
                bvb.rearrange("p (h d) -> p h d", h=HPC),
            )
        nc.vector.memset(v2_sb[:, :, :, HEAD_DIM:HEAD_DIM + 1], 1.0)

        # xh8: fp8 DoubleRow-interleaved normalized x (scaled columns)
        for c in range(CT):
            nc.vector.tensor_mul(
                xh8_sb[:, c // 2, c % 2, :], xs[c], scb)

    # ---------------- phase 2: attention main loop ----------------
    epool = stack.enter_context(tc.tile_pool(name="epool", bufs=2))
    espool = stack.enter_context(tc.tile_pool(name="espool", bufs=2))
    ptpool = stack.enter_context(tc.tile_pool(name="ptpool", bufs=4))
    smallpool = stack.enter_context(tc.tile_pool(name="smallpool", bufs=2))
    outstg = stack.enter_context(tc.tile_pool(name="outstg", bufs=4))
    mpool = (stack.enter_context(tc.tile_pool(name="mpool", bufs=2))
             if mask_d is not None else None)

    with tc.tile_pool(name="simps", bufs=1, space="PSUM") as simps, \
         tc.tile_pool(name="scps", bufs=1, space="PSUM") as scps, \
         tc.tile_pool(name="ctxps", bufs=1, space="PSUM") as ctxps, \
         tc.tile_pool(name="outps", bufs=1, space="PSUM") as outps:

        def emit_ctx(ctx_ps, kt, pt_pairs):
            for j in range(PAIRS):
                for hi in range(2):
                    h = 2 * j + hi
                    nc.tensor.matmul(
                        ctx_ps[h],
                        v2_sb[:, kt, h, :],
                        pt_pairs[j][:, hi, :],
                        start=(kt == 0),
                        stop=(kt == NKT - 1),
                        skip_group_check=True,
                    )

        def emit_out_proj(qb):
            # partial out-projection for this qb's rows
            for qt in range(qb * (QB // 128), (qb + 1) * (QB // 128)):
                for ob in range(NOB):
                    op = outps.tile([128, OB_W], F32, tag="op",
                                    name=f"op_{qt}_{ob}")
                    for j in range(PAIRS):
                        nc.tensor.matmul(
                            op,
                            ctxT2_sb[:, j, qt * 128:(qt + 1) * 128],
                            wo_sb[:, j, ob * OB_W:(ob + 1) * OB_W],
                            start=(j == 0),
                            stop=(j == PAIRS - 1),
                        )
                    ostg = outstg.tile([128, OB_W], F32, tag="ostg",
                                       name=f"ostg_{qt}_{ob}")
                    if (qt + ob) % 2 == 0:
                        nc.scalar.activation(out=ostg, in_=op, func=ACT_COPY)
                    else:
                        nc.vector.tensor_copy(ostg, op)
                    nc.sync.dma_start(
                        out=out_d[qt * 128:(qt + 1) * 128,
                                  ob * OB_W:(ob + 1) * OB_W],
                        in_=ostg,
                    )

        pending_out = None
        inv_ps2 = -1.0 / (XH_PRESCALE * XH_PRESCALE)
        for qb in range(NQB):
            qsl = slice(qb * QB, (qb + 1) * QB)
            ctx_ps = [ctxps.tile([HEAD_DIM + 1, QB], F32, tag=f"ctx{h}",
                                 name=f"ctx_{qb}_{h}")
                      for h in range(HPC)]
            pending = []
            for kt in range(NKT):
                ksl = slice(kt * 128, (kt + 1) * 128)
                # sim (fp8 DoubleRow): psum = PRESCALE^2 * gamma * sim
                sp = simps.tile([128, QB], F32, tag="sim",
                                name=f"sim_{qb}_{kt}")
                for cc in range(CC):
                    nc.tensor.matmul(
                        sp,
                        xh8_sb[:, cc, :, ksl],
                        xh8_sb[:, cc, :, qsl],
                        start=(cc == 0),
                        stop=(cc == CC - 1),
                        perf_mode=DR,
                    )
                e_t = epool.tile([128, QB], BF16, tag="E",
                                 name=f"E_{qb}_{kt}")
                nc.scalar.activation(out=e_t, in_=sp, func=ACT_EXP,
                                     scale=inv_ps2)
                if mask_d is not None:
                    m_sb = mpool.tile([128, QB], BF16, tag="msk")
                    nc.sync.dma_start(out=m_sb, in_=mask_d[ksl, qsl])
                    nc.vector.tensor_mul(e_t, e_t, m_sb)
                eb = e_t.unsqueeze(1).to_broadcast([128, 2, QB])

                pt_pairs = []
                for j in range(PAIRS):
                    sc_t = scps.tile([128, 2, QB], F32, tag="sc",
                                     name=f"sc_{qb}_{kt}_{j}")
                    for hi in range(2):
                        pr = slice(hi * 64, hi * 64 + 64)
                        nc.tensor.matmul(
                            sc_t[:, hi, :],
                            kT_sb[pr, j, ksl],
                            qT_sb[pr, j, qsl],
                            start=True,
                            stop=True,
                        )
                    es_t = espool.tile([128, 2, QB], BF16, tag=f"es{j}",
                                       name=f"es_{qb}_{kt}_{j}")
                    nc.scalar.activation(out=es_t, in_=sc_t, func=ACT_EXP)
                    pt = ptpool.tile([128, 2, QB], BF16, tag=f"pt{j}",
                                     name=f"pt_{qb}_{kt}_{j}")
                    nc.vector.tensor_mul(pt, es_t, eb)
                    pt_pairs.append(pt)
                if qb == 0 and kt == 0 and "dbg_E" in aps:
                    dE = outstg.tile([128, QB], F32, tag="dE")
                    nc.vector.tensor_copy(dE, e_t)
                    nc.sync.dma_start(out=aps["dbg_E"], in_=dE)
                    dES = outstg.tile([128, 2, QB], F32, tag="dES")
                    nc.vector.tensor_copy(dES, pt_pairs[0][:, :, :])
                    nc.sync.dma_start(out=aps["dbg_pt"], in_=dES)
                pending.append((kt, pt_pairs))
                if len(pending) > LAG:
                    k0, p0 = pending.pop(0)
                    emit_ctx(ctx_ps, k0, p0)
                if kt == 1 and pending_out is not None:
                    emit_out_proj(pending_out)
                    pending_out = None
            for k0, p0 in pending:
                emit_ctx(ctx_ps, k0, p0)

            # divisions: ctxT2 = ctx / sums (sums = ones-row at partition 64)
            if qb == 0 and "dbg_sums0" in aps:
                for h in range(HPC):
                    dS = outstg.tile([1, QB], F32, tag=f"dS{h}",
                                     name=f"dS_{h}")
                    nc.vector.tensor_copy(
                        dS, ctx_ps[h][HEAD_DIM:HEAD_DIM + 1, :])
                    nc.sync.dma_start(out=aps[f"dbg_sums{h}"], in_=dS)
            for h in range(HPC):
                j, hi = divmod(h, 2)
                s0 = smallpool.tile([1, QB], F32, tag="s0",
                                    name=f"s0_{qb}_{h}")
                nc.vector.tensor_copy(s0, ctx_ps[h][HEAD_DIM:HEAD_DIM + 1, :])
                r0 = smallpool.tile([1, QB], F32, tag="r0",
                                    name=f"r0_{qb}_{h}")
                nc.vector.reciprocal_approx_fast(r0, s0)
                rb = smallpool.tile([HEAD_DIM, QB], F32, tag="rb",
                                    name=f"rb_{qb}_{h}")
                nc.gpsimd.partition_broadcast(rb, r0, channels=HEAD_DIM)
                if qb == 0 and h == 0 and "dbg_r0" in aps:
                    dr0 = outstg.tile([1, QB], F32, tag="dr0")
                    nc.vector.tensor_copy(dr0, r0)
                    nc.sync.dma_start(out=aps["dbg_r0"], in_=dr0)
                    drb = outstg.tile([HEAD_DIM, QB], F32, tag="drb")
                    nc.vector.tensor_copy(drb, rb)
                    nc.sync.dma_start(out=aps["dbg_rb"], in_=drb)
                    dcr = outstg.tile([HEAD_DIM, QB], F32, tag="dcr")
                    nc.vector.tensor_copy(dcr, ctx_ps[h][0:HEAD_DIM, :])
                    nc.sync.dma_start(out=aps["dbg_ctxraw"], in_=dcr)
                nc.vector.tensor_mul(
                    ctxT2_sb[hi * 64:hi * 64 + 64, j, qsl],
                    ctx_ps[h][0:HEAD_DIM, :],
                    rb,
                )
            if qb == 0 and "dbg_ctxT2" in aps:
                dC = outstg.tile([128, QB], F32, tag="dC")
                nc.vector.tensor_copy(dC, ctxT2_sb[:, 0, 0:QB])
                nc.sync.dma_start(out=aps["dbg_ctxT2"], in_=dC)
            pending_out = qb
        emit_out_proj(pending_out)

    stack.close()


def build_nc(*, S_=S, C_=HIDDEN, QB=512, with_mask=False,
             enable_asserts=False):
    nc = bacc.Bacc(
        "TRN2", target_bir_lowering=False, debug=False,
        enable_asserts=enable_asserts,
    )
    D2 = HPC * HEAD_DIM
    aps = {}
    aps["xT"] = nc.dram_tensor("xT", [C_, S_], F32R, kind="ExternalInput").ap()
    aps["scale"] = nc.dram_tensor("scale", [1, S_], F32R, kind="ExternalInput").ap()
    for n in ("wq", "wk", "wv"):
        aps[n] = nc.dram_tensor(n, [C_, D2], F32R, kind="ExternalInput").ap()
    aps["wo"] = nc.dram_tensor("wo", [D2, C_], F32, kind="ExternalInput").ap()
    for n in ("bq", "bk", "bv"):
        aps[n] = nc.dram_tensor(n, [D2, 1], F32, kind="ExternalInput").ap()
    if with_mask:
        aps["maskmul"] = nc.dram_tensor(
            "maskmul", [S_, S_], BF16, kind="ExternalInput").ap()
    if os.environ.get("KDBG"):
        for n, shp in (("dbg_E", [128, 512]), ("dbg_pt", [128, 2, 512]),
                       ("dbg_sums0", [1, 512]), ("dbg_sums1", [1, 512]),
                       ("dbg_sums2", [1, 512]), ("dbg_sums3", [1, 512]),
                       ("dbg_ctxT2", [128, 512]), ("dbg_r0", [1, 512]),
                       ("dbg_rb", [64, 512]), ("dbg_ctxraw", [64, 512])):
            aps[n] = nc.dram_tensor(n, shp, F32, kind="ExternalOutput").ap()
    aps["out"] = nc.dram_tensor("out", [S_, C_], F32, kind="ExternalOutput").ap()

    with tile.TileContext(nc) as tc:
        emit_kernel(tc, aps, S_=S_, C_=C_, QB=QB)
    nc.compile()
    return nc


def host_prepare(x, attn_mask, Wq, bq, Wk, bk, Wv, bv, Wo, bo, *,
                 S_=S, C_=HIDDEN, n_cores=N_CORES):
    """Build the per-core input maps. Returns (in_maps, with_mask)."""
    x = np.asarray(x, np.float32)
    B_ = x.shape[0]
    groups = n_cores // B_
    Wq = np.asarray(Wq, np.float32); Wk = np.asarray(Wk, np.float32)
    Wv = np.asarray(Wv, np.float32); Wo = np.asarray(Wo, np.float32)
    bq = np.asarray(bq, np.float32); bk = np.asarray(bk, np.float32)
    bv = np.asarray(bv, np.float32)

    inv_sqrt_d = 1.0 / math.sqrt(HEAD_DIM)
    WqT = np.ascontiguousarray((Wq * inv_sqrt_d).T)  # [C, C] in->out
    WkT = np.ascontiguousarray(Wk.T)
    WvT = np.ascontiguousarray(Wv.T)
    WoT = np.ascontiguousarray(Wo.T)                 # [C(c), C(o)]
    bq = bq * inv_sqrt_d

    mask = np.asarray(attn_mask)
    with_mask = bool(mask.any())
    maskmul = None
    if with_mask:
        import ml_dtypes
        # reference: where(mask, -inf) -> multiplicative 0/1 on exp values
        # device layout: maskmul[k, q] multiplies exp-scores^T
        maskmul = np.where(mask.T, 0.0, 1.0).astype(ml_dtypes.bfloat16)
        maskmul = np.ascontiguousarray(maskmul)

    in_maps = []
    for core in range(n_cores):
        b, g = divmod(core, groups)
        xb = x[b]                                   # [S, C]
        xT = np.ascontiguousarray(xb.T)             # [C, S]
        norms = np.linalg.norm(xb, axis=1)          # [S]
        scale = (XH_PRESCALE * math.sqrt(GAMMA)
                 / np.maximum(norms, 1e-12)).astype(np.float32)
        D2 = HPC * HEAD_DIM
        ch = slice(g * D2, (g + 1) * D2)
        m = {
            "xT": xT,
            "scale": scale.reshape(1, S_),
            "wq": np.ascontiguousarray(WqT[:, ch]),
            "wk": np.ascontiguousarray(WkT[:, ch]),
            "wv": np.ascontiguousarray(WvT[:, ch]),
            "wo": np.ascontiguousarray(WoT[ch, :]),
            "bq": np.ascontiguousarray(bq[ch]).reshape(-1, 1),
            "bk": np.ascontiguousarray(bk[ch]).reshape(-1, 1),
            "bv": np.ascontiguousarray(bv[ch]).reshape(-1, 1),
        }
        if with_mask:
            m["maskmul"] = maskmul
        in_maps.append(m)
    return in_maps, with_mask


_NC_CACHE = {}


def _get_nc(with_mask):
    key = with_mask
    if key not in _NC_CACHE:
        _NC_CACHE[key] = build_nc(with_mask=with_mask)
    return _NC_CACHE[key]


LAST_RESULTS = None


def kernel(**inputs):
    global LAST_RESULTS
    in_maps, with_mask = host_prepare(
        inputs["x"], inputs["attn_mask"],
        inputs["Wq"], inputs["bq"], inputs["Wk"], inputs["bk"],
        inputs["Wv"], inputs["bv"], inputs["Wo"], inputs["bo"],
    )
    nc = _get_nc(with_mask)
    res = run_bass_kernel_spmd(nc, in_maps, core_ids=list(range(N_CORES)))
    LAST_RESULTS = res
    bo = np.asarray(inputs["bo"], np.float32)
    out = np.zeros((B, S, HIDDEN), np.float32)
    groups = N_CORES // B
    for core in range(N_CORES):
        b = core // groups
        out[b] += np.asarray(res.results[core]["out"], np.float32)
    out += bo[None, None, :]
    return out


# revision 8
# speedup vs baseline: 1.5899x; 1.0836x over previous
"""DiversityAttention on 8 TRN2 NeuronCores (Bass/Tile), v3.

Sharding: data-parallel over batch (B=2) x tensor-parallel over heads
(16 heads -> 4 groups of 4). core = (b, g), b = core // 4, g = core % 4.
Each core computes full attention for its 4 heads over its batch and a
partial out-projection [S, HIDDEN]; the host sums the 4 partials per
batch and adds bo.

Keys-on-partitions orientation, all-bf16 matmuls except the fp8
DoubleRow similarity:
  qT = (Wq/8 @ x^T + bq/8)  [64h, S] bf16    (1/sqrt(dh) folded on host)
  kT = (Wk   @ x^T + bk)    [64h, S] bf16
  V  = x @ WvT + bv directly in [keys, dh] layout, bf16 + ones column
  xh8 = fp8e4(x^T * 64*sqrt(gamma)/||x||)  (DoubleRow-interleaved)
  per (qb, kt):
     sim_psum = xh8^T xh8 (fp8 DoubleRow) = 4096*gamma*sim
     E = exp(-sim_psum/4096) (ACT, scale folded)          [128, QB] bf16
     scores_psum = kT^T qT per head pair (row-packed 64-contraction)
     es = exp(scores_psum) (ACT reads PSUM)               bf16
     pt = es * E (DVE bf16 2x)
     ctx_psum[65, QB] += [V|1]^T pt  (per head, ones row = softmax sums)
  sim/scores/out-proj PSUM tiles rotate through two 2-bank tags so the
  exp of one tile overlaps matmuls into the other; ctx holds 4 banks.
  at qb end: copy sums -> recip_approx_fast -> gpsimd broadcast -> DVE
  mul gives ctxT2 bf16; qb's out-projection is spread over kt=1..4 of
  the next qb so the PE never idles at block boundaries.
"""

import math
import os
import sys

import numpy as np

for _p in ("/opt/trn_rl_repo",):
    if _p not in sys.path and os.path.isdir(_p):
        sys.path.insert(0, _p)

os.environ.setdefault("MYCRO_LOCAL_CACHE", "1")

import concourse.bass as bass
import concourse.tile as tile
from concourse import bacc, mybir
from concourse.bass_utils import run_bass_kernel_spmd


def _install_ntff_hook():
    """Provide antenv.axon_hooks (NTFF profiling registry) if the image
    lacks it, mirroring trn_agent_boot's ctypes hook. No-op on failure."""
    try:
        import antenv.axon_hooks  # noqa: F401
        return
    except ImportError:
        pass
    try:
        import contextlib
        import ctypes
        import types

        so_path = "/opt/axon/libaxon_pjrt.so"
        if not os.path.exists(so_path):
            return
        lib = ctypes.CDLL(so_path)
        if not hasattr(lib, "axon_start_nrt_profile"):
            return
        lib.axon_start_nrt_profile.argtypes = [
            ctypes.POINTER(ctypes.c_int64), ctypes.c_size_t]
        lib.axon_start_nrt_profile.restype = ctypes.c_int64
        lib.axon_stop_nrt_profile.argtypes = [ctypes.c_char_p]
        lib.axon_stop_nrt_profile.restype = ctypes.c_int64

        @contextlib.contextmanager
        def _hook(output_dir, device_ids):
            import jax
            jax.devices()
            if device_ids:
                ids = (ctypes.c_int64 * len(device_ids))(*device_ids)
                rc = lib.axon_start_nrt_profile(ids, len(device_ids))
            else:
                rc = lib.axon_start_nrt_profile(None, 0)
            if rc != 0:
                raise RuntimeError(f"axon_start_nrt_profile rc={rc}")
            try:
                yield
            finally:
                n = lib.axon_stop_nrt_profile(str(output_dir).encode())
                print(f"ntff profile: {n} file(s) -> {output_dir}",
                      file=sys.stderr)

        mod = types.ModuleType("antenv.axon_hooks")
        _state = {"hook": _hook}
        mod.set_axon_ntff_profile_hook = lambda h: _state.__setitem__("hook", h)
        mod.get_axon_ntff_profile_hook = lambda: _state["hook"]
        sys.modules["antenv.axon_hooks"] = mod
        import antenv
        antenv.axon_hooks = mod
    except Exception:
        pass


_install_ntff_hook()

F32 = mybir.dt.float32
BF16 = mybir.dt.bfloat16
FP8 = mybir.dt.float8e4
ACT_EXP = mybir.ActivationFunctionType.Exp
ACT_COPY = mybir.ActivationFunctionType.Copy
DR = mybir.MatmulPerfMode.DoubleRow

# Problem constants (hardcoded per contract).
HIDDEN = 1024
HEADS = 16
HEAD_DIM = 64
GAMMA = 0.5
B, S = 2, 2048
N_CORES = 8
GROUPS = N_CORES // B  # head groups per batch
HPC = HEADS // GROUPS  # heads per core
PAIRS = HPC // 2
LAG = 3                # kt software-pipeline lag between pt and ctx matmul
XH_PRESCALE = 64.0     # fp8 prescale; sim psum = PRESCALE^2 * gamma * sim


def emit_kernel(tc, aps, *, S_, C_, QB):
    nc = tc.nc
    CT = C_ // 128          # contraction tiles over hidden
    CC = CT // 2            # fp8 DoubleRow chunks (256 rows each)
    NKT = S_ // 128         # key tiles
    NQB = S_ // QB          # query blocks
    PB = 512                # projection free-block width
    NPB = S_ // PB
    D2 = HPC * HEAD_DIM     # per-core projected channels

    xT_d = aps["xT"]; scale_d = aps["scale"]
    wq_d = aps["wq"]; wk_d = aps["wk"]; wv_d = aps["wv"]; wo_d = aps["wo"]
    bq_d = aps["bq"]; bk_d = aps["bk"]; bv_d = aps["bv"]
    out_d = aps["out"]
    mask_d = aps.get("maskmul")

    from contextlib import ExitStack
    stack = ExitStack()

    # --- persistent SBUF tensors ---
    proj = stack.enter_context(tc.tile_pool(name="proj", bufs=1))
    qT_sb = proj.tile([128, PAIRS, S_], BF16)      # head pairs on 64-halves
    kT_sb = proj.tile([128, PAIRS, S_], BF16)
    v2_sb = proj.tile([128, NKT, HPC, HEAD_DIM + 1], BF16)
    xh8_sb = proj.tile([128, CC, 2, S_], FP8)      # DoubleRow interleaved
    wo_sb = proj.tile([128, PAIRS, C_], BF16)
    ctxT2_sb = proj.tile([128, PAIRS, S_], BF16)

    # ---------------- phase 1: load + projections ----------------
    with tc.tile_pool(name="xsp", bufs=1) as xsp, \
         tc.tile_pool(name="wsp", bufs=1) as wsp, \
         tc.tile_pool(name="prjps", bufs=2, space="PSUM") as prjps, \
         tc.tile_pool(name="vps", bufs=2, space="PSUM") as vps:

        # wk first (consumption order: kT computed first), then x, then rest
        w_sb = {}
        for c in range(CT):
            wt = wsp.tile([128, D2], BF16, tag=f"wk{c}", name=f"wk_{c}")
            nc.sync.dma_start(out=wt, in_=wk_d[c * 128:(c + 1) * 128, :])
            w_sb[("wk", c)] = wt
        xs = []
        for c in range(CT):
            xt = xsp.tile([128, S_], BF16, tag=f"xs{c}", name=f"xs_{c}")
            nc.sync.dma_start(out=xt, in_=xT_d[c * 128:(c + 1) * 128, :])
            xs.append(xt)
        for wname, w_d in (("wq", wq_d), ("wv", wv_d)):
            for c in range(CT):
                wt = wsp.tile([128, D2], BF16, tag=f"{wname}{c}",
                              name=f"{wname}_{c}")
                nc.sync.dma_start(out=wt, in_=w_d[c * 128:(c + 1) * 128, :])
                w_sb[(wname, c)] = wt
        nc.sync.dma_start(
            out=wo_sb, in_=wo_d.rearrange("(j p) o -> p j o", p=128))

        b_sb = {}
        for bname, b_d in (("bq", bq_d), ("bk", bk_d), ("bv", bv_d)):
            bt = wsp.tile([128, PAIRS, 1], F32, tag=f"b{bname}")
            nc.sync.dma_start(
                out=bt, in_=b_d.rearrange("(j p) one -> p j one", p=128))
            b_sb[bname] = bt

        # normalization scale row -> broadcast to all partitions (gpsimd)
        scr = wsp.tile([1, S_], BF16, tag="scr")
        nc.sync.dma_start(out=scr, in_=scale_d)
        scb = wsp.tile([128, S_], BF16, tag="scb")
        nc.gpsimd.partition_broadcast(scb, scr, channels=128)

        # bv broadcast row for the V bias add ([1, D2] varies along free dim)
        bvr = wsp.tile([1, D2], F32, tag="bvr")
        nc.sync.dma_start(out=bvr, in_=bv_d.rearrange("d one -> one d"))
        bvb = wsp.tile([128, D2], F32, tag="bvb")
        nc.gpsimd.partition_broadcast(bvb, bvr, channels=128)

        # q/k projections: W tiles stationary, x moving; accumulate over c
        for wname, bname, dest in (("wk", "bk", kT_sb), ("wq", "bq", qT_sb)):
            for nb in range(NPB):
                pss = [prjps.tile([128, PB], F32, tag=f"prj{j}",
                                  name=f"prj_{wname}_{nb}_{j}")
                       for j in range(PAIRS)]
                for c in range(CT):
                    for j in range(PAIRS):
                        nc.tensor.matmul(
                            pss[j],
                            w_sb[(wname, c)][:, j * 128:(j + 1) * 128],
                            xs[c][:, nb * PB:(nb + 1) * PB],
                            start=(c == 0),
                            stop=(c == CT - 1),
                        )
                for j in range(PAIRS):
                    nc.vector.tensor_scalar_add(
                        dest[:, j, nb * PB:(nb + 1) * PB], pss[j],
                        b_sb[bname][:, j, :])

        # V directly in [keys, dh] layout: x tile stationary, WvT moving
        for st in range(NKT):
            vp = vps.tile([128, D2], F32, tag="vp", name=f"vp_{st}")
            for c in range(CT):
                nc.tensor.matmul(
                    vp,
                    xs[c][:, st * 128:(st + 1) * 128],
                    w_sb[("wv", c)],
                    start=(c == 0),
                    stop=(c == CT - 1),
                )
            nc.vector.tensor_add(
                v2_sb[:, st, :, 0:HEAD_DIM],
                vp.rearrange("p (h d) -> p h d", h=HPC),
                bvb.rearrange("p (h d) -> p h d", h=HPC),
            )
        nc.vector.memset(v2_sb[:, :, :, HEAD_DIM:HEAD_DIM + 1], 1.0)

        # xh8: fp8 DoubleRow-interleaved normalized x (scaled columns)
        for c in range(CT):
            nc.vector.tensor_mul(
                xh8_sb[:, c // 2, c % 2, :], xs[c], scb)

    # ---------------- phase 2: attention main loop ----------------
    epool = stack.enter_context(tc.tile_pool(name="epool", bufs=2))
    espool = stack.enter_context(tc.tile_pool(name="espool", bufs=2))
    ptpool = stack.enter_context(tc.tile_pool(name="ptpool", bufs=4))
    smallpool = stack.enter_context(tc.tile_pool(name="smallpool", bufs=2))
    outstg = stack.enter_context(tc.tile_pool(name="outstg", bufs=3))
    mpool = (stack.enter_context(tc.tile_pool(name="mpool", bufs=2))
             if mask_d is not None else None)

    with tc.tile_pool(name="mmps", bufs=1, space="PSUM") as mmps, \
         tc.tile_pool(name="ctxps", bufs=1, space="PSUM") as ctxps:

        mmctr = [0]

        def mm_tile(name):
            t = mmps.tile([128, 2, QB], F32, tag=f"T{mmctr[0] % 2}",
                          name=name)
            mmctr[0] += 1
            return t

        def emit_ctx(ctx_ps, kt, pt_pairs):
            for j in range(PAIRS):
                for hi in range(2):
                    h = 2 * j + hi
                    nc.tensor.matmul(
                        ctx_ps[h],
                        v2_sb[:, kt, h, :],
                        pt_pairs[j][:, hi, :],
                        start=(kt == 0),
                        stop=(kt == NKT - 1),
                        skip_group_check=True,
                    )

        def emit_out_qt(qt):
            # one query tile's partial out-projection: [128, 1024]
            op = mm_tile(f"op_{qt}")
            for ob in range(2):
                for j in range(PAIRS):
                    nc.tensor.matmul(
                        op[:, ob, :],
                        ctxT2_sb[:, j, qt * 128:(qt + 1) * 128],
                        wo_sb[:, j, ob * QB:(ob + 1) * QB],
                        start=(j == 0),
                        stop=(j == PAIRS - 1),
                    )
            ostg = outstg.tile([128, 2, QB], F32, tag="ostg",
                               name=f"ostg_{qt}")
            nc.vector.tensor_copy(ostg[:, 0, :], op[:, 0, :])
            nc.scalar.activation(out=ostg[:, 1, :], in_=op[:, 1, :],
                                 func=ACT_COPY)
            nc.sync.dma_start(
                out=out_d[qt * 128:(qt + 1) * 128, :],
                in_=ostg.rearrange("p a b -> p (a b)"),
            )

        pending_out = None
        inv_ps2 = -1.0 / (XH_PRESCALE * XH_PRESCALE)
        for qb in range(NQB):
            qsl = slice(qb * QB, (qb + 1) * QB)
            ctx_ps = [ctxps.tile([HEAD_DIM + 1, QB], F32, tag=f"ctx{h}",
                                 name=f"ctx_{qb}_{h}")
                      for h in range(HPC)]
            pending = []
            for kt in range(NKT):
                ksl = slice(kt * 128, (kt + 1) * 128)
                # sim (fp8 DoubleRow): psum = PRESCALE^2 * gamma * sim
                spt = mm_tile(f"sim_{qb}_{kt}")
                sp = spt[:, 0, :]
                for cc in range(CC):
                    nc.tensor.matmul(
                        sp,
                        xh8_sb[:, cc, :, ksl],
                        xh8_sb[:, cc, :, qsl],
                        start=(cc == 0),
                        stop=(cc == CC - 1),
                        perf_mode=DR,
                    )
                e_t = epool.tile([128, QB], BF16, tag="E",
                                 name=f"E_{qb}_{kt}")
                nc.scalar.activation(out=e_t, in_=sp, func=ACT_EXP,
                                     scale=inv_ps2)
                if mask_d is not None:
                    m_sb = mpool.tile([128, QB], BF16, tag="msk")
                    nc.sync.dma_start(out=m_sb, in_=mask_d[ksl, qsl])
                    nc.vector.tensor_mul(e_t, e_t, m_sb)
                eb = e_t.unsqueeze(1).to_broadcast([128, 2, QB])

                pt_pairs = []
                for j in range(PAIRS):
                    sc_t = mm_tile(f"sc_{qb}_{kt}_{j}")
                    for hi in range(2):
                        pr = slice(hi * 64, hi * 64 + 64)
                        nc.tensor.matmul(
                            sc_t[:, hi, :],
                            kT_sb[pr, j, ksl],
                            qT_sb[pr, j, qsl],
                            start=True,
                            stop=True,
                        )
                    es_t = espool.tile([128, 2, QB], BF16, tag=f"es{j}",
                                       name=f"es_{qb}_{kt}_{j}")
                    nc.scalar.activation(out=es_t, in_=sc_t, func=ACT_EXP)
                    pt = ptpool.tile([128, 2, QB], BF16, tag=f"pt{j}",
                                     name=f"pt_{qb}_{kt}_{j}")
                    nc.vector.tensor_mul(pt, es_t, eb)
                    pt_pairs.append(pt)
                pending.append((kt, pt_pairs))
                if len(pending) > LAG:
                    k0, p0 = pending.pop(0)
                    emit_ctx(ctx_ps, k0, p0)
                if 1 <= kt <= 4 and pending_out is not None:
                    emit_out_qt(pending_out * (QB // 128) + kt - 1)
                    if kt == 4:
                        pending_out = None
            for k0, p0 in pending:
                emit_ctx(ctx_ps, k0, p0)

            # divisions: ctxT2 = ctx / sums (sums = ones-row at partition 64)
            for h in range(HPC):
                j, hi = divmod(h, 2)
                s0 = smallpool.tile([1, QB], F32, tag="s0",
                                    name=f"s0_{qb}_{h}")
                nc.vector.tensor_copy(s0, ctx_ps[h][HEAD_DIM:HEAD_DIM + 1, :])
                r0 = smallpool.tile([1, QB], F32, tag="r0",
                                    name=f"r0_{qb}_{h}")
                nc.vector.reciprocal_approx_fast(r0, s0)
                rb = smallpool.tile([HEAD_DIM, QB], F32, tag="rb",
                                    name=f"rb_{qb}_{h}")
                nc.gpsimd.partition_broadcast(rb, r0, channels=HEAD_DIM)
                nc.vector.tensor_mul(
                    ctxT2_sb[hi * 64:hi * 64 + 64, j, qsl],
                    ctx_ps[h][0:HEAD_DIM, :],
                    rb,
                )
            pending_out = qb
        for qt in range(pending_out * (QB // 128),
                        (pending_out + 1) * (QB // 128)):
            emit_out_qt(qt)

    stack.close()


def build_nc(*, S_=S, C_=HIDDEN, QB=512, with_mask=False,
             enable_asserts=False):
    nc = bacc.Bacc(
        "TRN2", target_bir_lowering=False, debug=False,
        enable_asserts=enable_asserts,
    )
    D2 = HPC * HEAD_DIM
    aps = {}
    aps["xT"] = nc.dram_tensor("xT", [C_, S_], BF16, kind="ExternalInput").ap()
    aps["scale"] = nc.dram_tensor(
        "scale", [1, S_], BF16, kind="ExternalInput").ap()
    for n in ("wq", "wk", "wv"):
        aps[n] = nc.dram_tensor(n, [C_, D2], BF16, kind="ExternalInput").ap()
    aps["wo"] = nc.dram_tensor("wo", [D2, C_], BF16, kind="ExternalInput").ap()
    for n in ("bq", "bk", "bv"):
        aps[n] = nc.dram_tensor(n, [D2, 1], F32, kind="ExternalInput").ap()
    if with_mask:
        aps["maskmul"] = nc.dram_tensor(
            "maskmul", [S_, S_], BF16, kind="ExternalInput").ap()
    aps["out"] = nc.dram_tensor("out", [S_, C_], F32, kind="ExternalOutput").ap()

    with tile.TileContext(nc) as tc:
        emit_kernel(tc, aps, S_=S_, C_=C_, QB=QB)
    nc.compile()
    return nc


def host_prepare(x, attn_mask, Wq, bq, Wk, bk, Wv, bv, Wo, bo, *,
                 S_=S, C_=HIDDEN, n_cores=N_CORES):
    """Build the per-core input maps. Returns (in_maps, with_mask)."""
    import ml_dtypes
    BF = ml_dtypes.bfloat16
    x = np.asarray(x, np.float32)
    B_ = x.shape[0]
    groups = n_cores // B_
    Wq = np.asarray(Wq, np.float32); Wk = np.asarray(Wk, np.float32)
    Wv = np.asarray(Wv, np.float32); Wo = np.asarray(Wo, np.float32)
    bq = np.asarray(bq, np.float32); bk = np.asarray(bk, np.float32)
    bv = np.asarray(bv, np.float32)

    inv_sqrt_d = 1.0 / math.sqrt(HEAD_DIM)
    WqT = np.ascontiguousarray((Wq * inv_sqrt_d).T).astype(BF)
    WkT = np.ascontiguousarray(Wk.T).astype(BF)
    WvT = np.ascontiguousarray(Wv.T).astype(BF)
    WoT = np.ascontiguousarray(Wo.T).astype(BF)      # [C(c), C(o)]
    bq = bq * inv_sqrt_d

    mask = np.asarray(attn_mask)
    with_mask = bool(mask.any())
    maskmul = None
    if with_mask:
        # reference: where(mask, -inf) -> multiplicative 0/1 on exp values
        maskmul = np.where(mask.T, 0.0, 1.0).astype(BF)
        maskmul = np.ascontiguousarray(maskmul)

    in_maps = []
    for core in range(n_cores):
        b, g = divmod(core, groups)
        xb = x[b]                                   # [S, C]
        xT = np.ascontiguousarray(xb.T).astype(BF)  # [C, S]
        norms = np.linalg.norm(xb, axis=1)          # [S]
        scale = (XH_PRESCALE * math.sqrt(GAMMA)
                 / np.maximum(norms, 1e-12)).astype(BF)
        D2 = HPC * HEAD_DIM
        ch = slice(g * D2, (g + 1) * D2)
        m = {
            "xT": xT,
            "scale": scale.reshape(1, S_),
            "wq": np.ascontiguousarray(WqT[:, ch]),
            "wk": np.ascontiguousarray(WkT[:, ch]),
            "wv": np.ascontiguousarray(WvT[:, ch]),
            "wo": np.ascontiguousarray(WoT[ch, :]),
            "bq": np.ascontiguousarray(bq[ch]).reshape(-1, 1),
            "bk": np.ascontiguousarray(bk[ch]).reshape(-1, 1),
            "bv": np.ascontiguousarray(bv[ch]).reshape(-1, 1),
        }
        if with_mask:
            m["maskmul"] = maskmul
        in_maps.append(m)
    return in_maps, with_mask


_NC_CACHE = {}


def _get_nc(with_mask):
    key = with_mask
    if key not in _NC_CACHE:
        _NC_CACHE[key] = build_nc(with_mask=with_mask)
    return _NC_CACHE[key]


LAST_RESULTS = None


def kernel(**inputs):
    global LAST_RESULTS
    in_maps, with_mask = host_prepare(
        inputs["x"], inputs["attn_mask"],
        inputs["Wq"], inputs["bq"], inputs["Wk"], inputs["bk"],
        inputs["Wv"], inputs["bv"], inputs["Wo"], inputs["bo"],
    )
    nc = _get_nc(with_mask)
    res = run_bass_kernel_spmd(nc, in_maps, core_ids=list(range(N_CORES)))
    LAST_RESULTS = res
    bo = np.asarray(inputs["bo"], np.float32)
    out = np.zeros((B, S, HIDDEN), np.float32)
    groups = N_CORES // B
    for core in range(N_CORES):
        b = core // groups
        out[b] += np.asarray(res.results[core]["out"], np.float32)
    out += bo[None, None, :]
    return out


# revision 9
# speedup vs baseline: 1.6865x; 1.0608x over previous
"""DiversityAttention on 8 TRN2 NeuronCores (Bass/Tile), v3.

Sharding: data-parallel over batch (B=2) x tensor-parallel over heads
(16 heads -> 4 groups of 4). core = (b, g), b = core // 4, g = core % 4.
Each core computes full attention for its 4 heads over its batch and a
partial out-projection [S, HIDDEN]; the host sums the 4 partials per
batch and adds bo.

Keys-on-partitions orientation, all-bf16 matmuls except the fp8
DoubleRow similarity:
  qT = (Wq/8 @ x^T + bq/8)  [64h, S] bf16    (1/sqrt(dh) folded on host)
  kT = (Wk   @ x^T + bk)    [64h, S] bf16
  V  = x @ WvT + bv directly in [keys, dh] layout, bf16 + ones column
  xh8 = fp8e4(x^T * 64*sqrt(gamma)/||x||)  (DoubleRow-interleaved)
  per (qb, kt):
     sim_psum = xh8^T xh8 (fp8 DoubleRow) = 4096*gamma*sim
     E = exp(-sim_psum/4096) (ACT, scale folded)          [128, QB] bf16
     scores_psum = kT^T qT per head pair (row-packed 64-contraction)
     es = exp(scores_psum) (ACT reads PSUM)               bf16
     pt = es * E (DVE bf16 2x)
     ctx_psum[65, QB] += [V|1]^T pt  (per head, ones row = softmax sums)
  sim/scores/out-proj PSUM tiles rotate through two 2-bank tags so the
  exp of one tile overlaps matmuls into the other; ctx holds 4 banks.
  at qb end: copy sums -> recip_approx_fast -> gpsimd broadcast -> DVE
  mul gives ctxT2 bf16; qb's out-projection is spread over kt=1..4 of
  the next qb so the PE never idles at block boundaries.
"""

import math
import os
import sys

import numpy as np

for _p in ("/opt/trn_rl_repo",):
    if _p not in sys.path and os.path.isdir(_p):
        sys.path.insert(0, _p)

os.environ.setdefault("MYCRO_LOCAL_CACHE", "1")

import concourse.bass as bass
import concourse.tile as tile
from concourse import bacc, mybir
from concourse.bass_utils import run_bass_kernel_spmd


def _install_ntff_hook():
    """Provide antenv.axon_hooks (NTFF profiling registry) if the image
    lacks it, mirroring trn_agent_boot's ctypes hook. No-op on failure."""
    try:
        import antenv.axon_hooks  # noqa: F401
        return
    except ImportError:
        pass
    try:
        import contextlib
        import ctypes
        import types

        so_path = "/opt/axon/libaxon_pjrt.so"
        if not os.path.exists(so_path):
            return
        lib = ctypes.CDLL(so_path)
        if not hasattr(lib, "axon_start_nrt_profile"):
            return
        lib.axon_start_nrt_profile.argtypes = [
            ctypes.POINTER(ctypes.c_int64), ctypes.c_size_t]
        lib.axon_start_nrt_profile.restype = ctypes.c_int64
        lib.axon_stop_nrt_profile.argtypes = [ctypes.c_char_p]
        lib.axon_stop_nrt_profile.restype = ctypes.c_int64

        @contextlib.contextmanager
        def _hook(output_dir, device_ids):
            import jax
            jax.devices()
            if device_ids:
                ids = (ctypes.c_int64 * len(device_ids))(*device_ids)
                rc = lib.axon_start_nrt_profile(ids, len(device_ids))
            else:
                rc = lib.axon_start_nrt_profile(None, 0)
            if rc != 0:
                raise RuntimeError(f"axon_start_nrt_profile rc={rc}")
            try:
                yield
            finally:
                n = lib.axon_stop_nrt_profile(str(output_dir).encode())
                print(f"ntff profile: {n} file(s) -> {output_dir}",
                      file=sys.stderr)

        mod = types.ModuleType("antenv.axon_hooks")
        _state = {"hook": _hook}
        mod.set_axon_ntff_profile_hook = lambda h: _state.__setitem__("hook", h)
        mod.get_axon_ntff_profile_hook = lambda: _state["hook"]
        sys.modules["antenv.axon_hooks"] = mod
        import antenv
        antenv.axon_hooks = mod
    except Exception:
        pass


_install_ntff_hook()

F32 = mybir.dt.float32
BF16 = mybir.dt.bfloat16
FP8 = mybir.dt.float8e4
ACT_EXP = mybir.ActivationFunctionType.Exp
ACT_COPY = mybir.ActivationFunctionType.Copy
DR = mybir.MatmulPerfMode.DoubleRow

# Problem constants (hardcoded per contract).
HIDDEN = 1024
HEADS = 16
HEAD_DIM = 64
GAMMA = 0.5
B, S = 2, 2048
N_CORES = 8
GROUPS = N_CORES // B  # head groups per batch
HPC = HEADS // GROUPS  # heads per core
PAIRS = HPC // 2
LAG = 3                # kt software-pipeline lag between pt and ctx matmul
XH_PRESCALE = 64.0     # fp8 prescale; sim psum = PRESCALE^2 * gamma * sim


def emit_kernel(tc, aps, *, S_, C_, QB):
    nc = tc.nc
    CT = C_ // 128          # contraction tiles over hidden
    CC = CT // 2            # fp8 DoubleRow chunks (256 rows each)
    NKT = S_ // 128         # key tiles
    NQB = S_ // QB          # query blocks
    PB = 512                # projection free-block width
    NPB = S_ // PB
    D2 = HPC * HEAD_DIM     # per-core projected channels

    xT_d = aps["xT"]; scale_d = aps["scale"]
    wq_d = aps["wq"]; wk_d = aps["wk"]; wv_d = aps["wv"]; wo_d = aps["wo"]
    bq_d = aps["bq"]; bk_d = aps["bk"]; bv_d = aps["bv"]
    out_d = aps["out"]
    mask_d = aps.get("maskmul")

    from contextlib import ExitStack
    stack = ExitStack()

    # --- persistent SBUF tensors ---
    proj = stack.enter_context(tc.tile_pool(name="proj", bufs=1))
    qT_sb = proj.tile([128, PAIRS, S_], BF16)      # head pairs on 64-halves
    kT_sb = proj.tile([128, PAIRS, S_], BF16)
    v2_sb = proj.tile([128, NKT, HPC, HEAD_DIM + 1], BF16)
    xh8_sb = proj.tile([128, CC, 2, S_], FP8)      # DoubleRow interleaved
    wo_sb = proj.tile([128, PAIRS, C_], BF16)
    ctxT2_sb = proj.tile([128, PAIRS, S_], BF16)

    # ---------------- phase 1: load + projections ----------------
    with tc.tile_pool(name="xsp", bufs=1) as xsp, \
         tc.tile_pool(name="wsp", bufs=1) as wsp, \
         tc.tile_pool(name="prjps", bufs=2, space="PSUM") as prjps, \
         tc.tile_pool(name="vps", bufs=2, space="PSUM") as vps:

        # wk first (consumption order: kT computed first), then x, then rest
        w_sb = {}
        for c in range(CT):
            wt = wsp.tile([128, D2], BF16, tag=f"wk{c}", name=f"wk_{c}")
            nc.sync.dma_start(out=wt, in_=wk_d[c * 128:(c + 1) * 128, :])
            w_sb[("wk", c)] = wt
        xs = []
        for c in range(CT):
            xt = xsp.tile([128, S_], BF16, tag=f"xs{c}", name=f"xs_{c}")
            nc.sync.dma_start(out=xt, in_=xT_d[c * 128:(c + 1) * 128, :])
            xs.append(xt)
        for wname, w_d in (("wq", wq_d), ("wv", wv_d)):
            for c in range(CT):
                wt = wsp.tile([128, D2], BF16, tag=f"{wname}{c}",
                              name=f"{wname}_{c}")
                nc.sync.dma_start(out=wt, in_=w_d[c * 128:(c + 1) * 128, :])
                w_sb[(wname, c)] = wt
        nc.sync.dma_start(
            out=wo_sb, in_=wo_d.rearrange("(j p) o -> p j o", p=128))

        b_sb = {}
        for bname, b_d in (("bq", bq_d), ("bk", bk_d), ("bv", bv_d)):
            bt = wsp.tile([128, PAIRS, 1], F32, tag=f"b{bname}")
            nc.sync.dma_start(
                out=bt, in_=b_d.rearrange("(j p) one -> p j one", p=128))
            b_sb[bname] = bt

        # normalization scale row -> broadcast to all partitions (gpsimd)
        scr = wsp.tile([1, S_], BF16, tag="scr")
        nc.sync.dma_start(out=scr, in_=scale_d)
        scb = wsp.tile([128, S_], BF16, tag="scb")
        nc.gpsimd.partition_broadcast(scb, scr, channels=128)

        # bv broadcast row for the V bias add ([1, D2] varies along free dim)
        bvr = wsp.tile([1, D2], F32, tag="bvr")
        nc.sync.dma_start(out=bvr, in_=bv_d.rearrange("d one -> one d"))
        bvb = wsp.tile([128, D2], F32, tag="bvb")
        nc.gpsimd.partition_broadcast(bvb, bvr, channels=128)

        # q/k projections: W tiles stationary, x moving; accumulate over c
        for wname, bname, dest in (("wk", "bk", kT_sb), ("wq", "bq", qT_sb)):
            for nb in range(NPB):
                pss = [prjps.tile([128, PB], F32, tag=f"prj{j}",
                                  name=f"prj_{wname}_{nb}_{j}")
                       for j in range(PAIRS)]
                for c in range(CT):
                    for j in range(PAIRS):
                        nc.tensor.matmul(
                            pss[j],
                            w_sb[(wname, c)][:, j * 128:(j + 1) * 128],
                            xs[c][:, nb * PB:(nb + 1) * PB],
                            start=(c == 0),
                            stop=(c == CT - 1),
                        )
                for j in range(PAIRS):
                    nc.vector.tensor_scalar_add(
                        dest[:, j, nb * PB:(nb + 1) * PB], pss[j],
                        b_sb[bname][:, j, :])

        # V directly in [keys, dh] layout: x tile stationary, WvT moving
        for st in range(NKT):
            vp = vps.tile([128, D2], F32, tag="vp", name=f"vp_{st}")
            for c in range(CT):
                nc.tensor.matmul(
                    vp,
                    xs[c][:, st * 128:(st + 1) * 128],
                    w_sb[("wv", c)],
                    start=(c == 0),
                    stop=(c == CT - 1),
                )
            nc.vector.tensor_add(
                v2_sb[:, st, :, 0:HEAD_DIM],
                vp.rearrange("p (h d) -> p h d", h=HPC),
                bvb.rearrange("p (h d) -> p h d", h=HPC),
            )
        nc.vector.memset(v2_sb[:, :, :, HEAD_DIM:HEAD_DIM + 1], 1.0)

        # xh8: fp8 DoubleRow-interleaved normalized x (scaled columns)
        for c in range(CT):
            nc.vector.tensor_mul(
                xh8_sb[:, c // 2, c % 2, :], xs[c], scb)

    # ---------------- phase 2: attention main loop ----------------
    epool = stack.enter_context(tc.tile_pool(name="epool", bufs=2))
    espool = stack.enter_context(tc.tile_pool(name="espool", bufs=2))
    ptpool = stack.enter_context(tc.tile_pool(name="ptpool", bufs=4))
    smallpool = stack.enter_context(tc.tile_pool(name="smallpool", bufs=2))
    outstg = stack.enter_context(tc.tile_pool(name="outstg", bufs=3))
    mpool = (stack.enter_context(tc.tile_pool(name="mpool", bufs=2))
             if mask_d is not None else None)

    with tc.tile_pool(name="mmps", bufs=1, space="PSUM") as mmps, \
         tc.tile_pool(name="ctxps", bufs=1, space="PSUM") as ctxps:

        mmctr = [0]

        def mm_tile(name):
            t = mmps.tile([128, 2, QB], F32, tag=f"T{mmctr[0] % 2}",
                          name=name)
            mmctr[0] += 1
            return t

        def emit_ctx(ctx_ps, kt, pt_pairs):
            for j in range(PAIRS):
                for hi in range(2):
                    h = 2 * j + hi
                    nc.tensor.matmul(
                        ctx_ps[h],
                        v2_sb[:, kt, h, :],
                        pt_pairs[j][:, hi, :],
                        start=(kt == 0),
                        stop=(kt == NKT - 1),
                        skip_group_check=True,
                    )

        def emit_out_qt(qt):
            # one query tile's partial out-projection: [128, 1024]
            op = mm_tile(f"op_{qt}")
            for ob in range(2):
                for j in range(PAIRS):
                    nc.tensor.matmul(
                        op[:, ob, :],
                        ctxT2_sb[:, j, qt * 128:(qt + 1) * 128],
                        wo_sb[:, j, ob * QB:(ob + 1) * QB],
                        start=(j == 0),
                        stop=(j == PAIRS - 1),
                    )
            ostg = outstg.tile([128, 2, QB], F32, tag="ostg",
                               name=f"ostg_{qt}")
            nc.vector.tensor_copy(ostg[:, 0, :], op[:, 0, :])
            nc.scalar.activation(out=ostg[:, 1, :], in_=op[:, 1, :],
                                 func=ACT_COPY)
            nc.sync.dma_start(
                out=out_d[qt * 128:(qt + 1) * 128, :],
                in_=ostg.rearrange("p a b -> p (a b)"),
            )

        def emit_divisions(qb, ctx_ps):
            qsl = slice(qb * QB, (qb + 1) * QB)
            for h in range(HPC):
                j, hi = divmod(h, 2)
                s0 = smallpool.tile([1, QB], F32, tag="s0",
                                    name=f"s0_{qb}_{h}")
                nc.vector.tensor_copy(s0, ctx_ps[h][HEAD_DIM:HEAD_DIM + 1, :])
                r0 = smallpool.tile([1, QB], F32, tag="r0",
                                    name=f"r0_{qb}_{h}")
                nc.vector.reciprocal_approx_fast(r0, s0)
                rb = smallpool.tile([HEAD_DIM, QB], F32, tag="rb",
                                    name=f"rb_{qb}_{h}")
                nc.gpsimd.partition_broadcast(rb, r0, channels=HEAD_DIM)
                nc.vector.tensor_mul(
                    ctxT2_sb[hi * 64:hi * 64 + 64, j, qsl],
                    ctx_ps[h][0:HEAD_DIM, :],
                    rb,
                )

        pending_out = None
        leftover = None    # (prev_qb, prev_ctx_ps, undrained pops)
        inv_ps2 = -1.0 / (XH_PRESCALE * XH_PRESCALE)
        for qb in range(NQB):
            qsl = slice(qb * QB, (qb + 1) * QB)
            ctx_ps = [ctxps.tile([HEAD_DIM + 1, QB], F32, tag=f"ctx{h}",
                                 name=f"ctx_{qb}_{h}")
                      for h in range(HPC)]
            pending = []
            for kt in range(NKT):
                ksl = slice(kt * 128, (kt + 1) * 128)
                # sim (fp8 DoubleRow): psum = PRESCALE^2 * gamma * sim
                spt = mm_tile(f"sim_{qb}_{kt}")
                sp = spt[:, 0, :]
                for cc in range(CC):
                    nc.tensor.matmul(
                        sp,
                        xh8_sb[:, cc, :, ksl],
                        xh8_sb[:, cc, :, qsl],
                        start=(cc == 0),
                        stop=(cc == CC - 1),
                        perf_mode=DR,
                    )
                e_t = epool.tile([128, QB], BF16, tag="E",
                                 name=f"E_{qb}_{kt}")
                nc.scalar.activation(out=e_t, in_=sp, func=ACT_EXP,
                                     scale=inv_ps2)
                if mask_d is not None:
                    m_sb = mpool.tile([128, QB], BF16, tag="msk")
                    nc.sync.dma_start(out=m_sb, in_=mask_d[ksl, qsl])
                    nc.vector.tensor_mul(e_t, e_t, m_sb)
                eb = e_t.unsqueeze(1).to_broadcast([128, 2, QB])

                pt_pairs = []
                for j in range(PAIRS):
                    sc_t = mm_tile(f"sc_{qb}_{kt}_{j}")
                    for hi in range(2):
                        pr = slice(hi * 64, hi * 64 + 64)
                        nc.tensor.matmul(
                            sc_t[:, hi, :],
                            kT_sb[pr, j, ksl],
                            qT_sb[pr, j, qsl],
                            start=True,
                            stop=True,
                        )
                    es_t = espool.tile([128, 2, QB], BF16, tag=f"es{j}",
                                       name=f"es_{qb}_{kt}_{j}")
                    nc.scalar.activation(out=es_t, in_=sc_t, func=ACT_EXP)
                    pt = ptpool.tile([128, 2, QB], BF16, tag=f"pt{j}",
                                     name=f"pt_{qb}_{kt}_{j}")
                    nc.vector.tensor_mul(pt, es_t, eb)
                    pt_pairs.append(pt)
                pending.append((kt, pt_pairs))
                # drain the previous qb's leftover pops, then its divisions
                if leftover is not None and kt <= 2:
                    lqb, lctx, lpend = leftover
                    k0, p0 = lpend.pop(0)
                    emit_ctx(lctx, k0, p0)
                    if not lpend:
                        emit_divisions(lqb, lctx)
                        pending_out = lqb
                        leftover = None
                if len(pending) > LAG:
                    k0, p0 = pending.pop(0)
                    emit_ctx(ctx_ps, k0, p0)
                if 4 <= kt <= 7 and pending_out is not None:
                    emit_out_qt(pending_out * (QB // 128) + kt - 4)
                    if kt == 7:
                        pending_out = None
            leftover = (qb, ctx_ps, pending)
        lqb, lctx, lpend = leftover
        for k0, p0 in lpend:
            emit_ctx(lctx, k0, p0)
        emit_divisions(lqb, lctx)
        for qt in range(lqb * (QB // 128), (lqb + 1) * (QB // 128)):
            emit_out_qt(qt)

    stack.close()


def build_nc(*, S_=S, C_=HIDDEN, QB=512, with_mask=False,
             enable_asserts=False):
    nc = bacc.Bacc(
        "TRN2", target_bir_lowering=False, debug=False,
        enable_asserts=enable_asserts,
    )
    D2 = HPC * HEAD_DIM
    aps = {}
    aps["xT"] = nc.dram_tensor("xT", [C_, S_], BF16, kind="ExternalInput").ap()
    aps["scale"] = nc.dram_tensor(
        "scale", [1, S_], BF16, kind="ExternalInput").ap()
    for n in ("wq", "wk", "wv"):
        aps[n] = nc.dram_tensor(n, [C_, D2], BF16, kind="ExternalInput").ap()
    aps["wo"] = nc.dram_tensor("wo", [D2, C_], BF16, kind="ExternalInput").ap()
    for n in ("bq", "bk", "bv"):
        aps[n] = nc.dram_tensor(n, [D2, 1], F32, kind="ExternalInput").ap()
    if with_mask:
        aps["maskmul"] = nc.dram_tensor(
            "maskmul", [S_, S_], BF16, kind="ExternalInput").ap()
    aps["out"] = nc.dram_tensor("out", [S_, C_], F32, kind="ExternalOutput").ap()

    with tile.TileContext(nc) as tc:
        emit_kernel(tc, aps, S_=S_, C_=C_, QB=QB)
    nc.compile()
    return nc


def host_prepare(x, attn_mask, Wq, bq, Wk, bk, Wv, bv, Wo, bo, *,
                 S_=S, C_=HIDDEN, n_cores=N_CORES):
    """Build the per-core input maps. Returns (in_maps, with_mask)."""
    import ml_dtypes
    BF = ml_dtypes.bfloat16
    x = np.asarray(x, np.float32)
    B_ = x.shape[0]
    groups = n_cores // B_
    Wq = np.asarray(Wq, np.float32); Wk = np.asarray(Wk, np.float32)
    Wv = np.asarray(Wv, np.float32); Wo = np.asarray(Wo, np.float32)
    bq = np.asarray(bq, np.float32); bk = np.asarray(bk, np.float32)
    bv = np.asarray(bv, np.float32)

    inv_sqrt_d = 1.0 / math.sqrt(HEAD_DIM)
    WqT = np.ascontiguousarray((Wq * inv_sqrt_d).T).astype(BF)
    WkT = np.ascontiguousarray(Wk.T).astype(BF)
    WvT = np.ascontiguousarray(Wv.T).astype(BF)
    WoT = np.ascontiguousarray(Wo.T).astype(BF)      # [C(c), C(o)]
    bq = bq * inv_sqrt_d

    mask = np.asarray(attn_mask)
    with_mask = bool(mask.any())
    maskmul = None
    if with_mask:
        # reference: where(mask, -inf) -> multiplicative 0/1 on exp values
        maskmul = np.where(mask.T, 0.0, 1.0).astype(BF)
        maskmul = np.ascontiguousarray(maskmul)

    in_maps = []
    for core in range(n_cores):
        b, g = divmod(core, groups)
        xb = x[b]                                   # [S, C]
        xT = np.ascontiguousarray(xb.T).astype(BF)  # [C, S]
        norms = np.linalg.norm(xb, axis=1)          # [S]
        scale = (XH_PRESCALE * math.sqrt(GAMMA)
                 / np.maximum(norms, 1e-12)).astype(BF)
        D2 = HPC * HEAD_DIM
        ch = slice(g * D2, (g + 1) * D2)
        m = {
            "xT": xT,
            "scale": scale.reshape(1, S_),
            "wq": np.ascontiguousarray(WqT[:, ch]),
            "wk": np.ascontiguousarray(WkT[:, ch]),
            "wv": np.ascontiguousarray(WvT[:, ch]),
            "wo": np.ascontiguousarray(WoT[ch, :]),
            "bq": np.ascontiguousarray(bq[ch]).reshape(-1, 1),
            "bk": np.ascontiguousarray(bk[ch]).reshape(-1, 1),
            "bv": np.ascontiguousarray(bv[ch]).reshape(-1, 1),
        }
        if with_mask:
            m["maskmul"] = maskmul
        in_maps.append(m)
    return in_maps, with_mask


_NC_CACHE = {}


def _get_nc(with_mask):
    key = with_mask
    if key not in _NC_CACHE:
        _NC_CACHE[key] = build_nc(with_mask=with_mask)
    return _NC_CACHE[key]


LAST_RESULTS = None


def kernel(**inputs):
    global LAST_RESULTS
    in_maps, with_mask = host_prepare(
        inputs["x"], inputs["attn_mask"],
        inputs["Wq"], inputs["bq"], inputs["Wk"], inputs["bk"],
        inputs["Wv"], inputs["bv"], inputs["Wo"], inputs["bo"],
    )
    nc = _get_nc(with_mask)
    res = run_bass_kernel_spmd(nc, in_maps, core_ids=list(range(N_CORES)))
    LAST_RESULTS = res
    bo = np.asarray(inputs["bo"], np.float32)
    out = np.zeros((B, S, HIDDEN), np.float32)
    groups = N_CORES // B
    for core in range(N_CORES):
        b = core // groups
        out[b] += np.asarray(res.results[core]["out"], np.float32)
    out += bo[None, None, :]
    return out


# revision 14
# speedup vs baseline: 1.8134x; 1.0752x over previous
"""DiversityAttention on 8 TRN2 NeuronCores (Bass/Tile), v3.

Sharding: data-parallel over batch (B=2) x tensor-parallel over heads
(16 heads -> 4 groups of 4). core = (b, g), b = core // 4, g = core % 4.
Each core computes full attention for its 4 heads over its batch and a
partial out-projection [S, HIDDEN]; the host sums the 4 partials per
batch and adds bo.

Keys-on-partitions orientation, all-bf16 matmuls except the fp8
DoubleRow similarity:
  qT = (Wq/8 @ x^T + bq/8)  [64h, S] bf16    (1/sqrt(dh) folded on host)
  kT = (Wk   @ x^T + bk)    [64h, S] bf16
  V  = x @ WvT + bv directly in [keys, dh] layout, bf16 + ones column
  xh8 = fp8e4(x^T * 64*sqrt(gamma)/||x||)  (DoubleRow-interleaved)
  per (qb, kt):
     sim_psum = xh8^T xh8 (fp8 DoubleRow) = 4096*gamma*sim
     E = exp(-sim_psum/4096) (ACT, scale folded)          [128, QB] bf16
     scores_psum = kT^T qT per head pair (row-packed 64-contraction)
     es = exp(scores_psum) (ACT reads PSUM)               bf16
     pt = es * E (DVE bf16 2x)
     ctx_psum[65, QB] += [V|1]^T pt  (per head, ones row = softmax sums)
  sim/scores/out-proj PSUM tiles rotate through two 2-bank tags so the
  exp of one tile overlaps matmuls into the other; ctx holds 4 banks.
  at qb end: copy sums -> recip_approx_fast -> gpsimd broadcast -> DVE
  mul gives ctxT2 bf16; qb's out-projection is spread over kt=1..4 of
  the next qb so the PE never idles at block boundaries.
"""

import math
import os
import sys

import numpy as np

for _p in ("/opt/trn_rl_repo",):
    if _p not in sys.path and os.path.isdir(_p):
        sys.path.insert(0, _p)

os.environ.setdefault("MYCRO_LOCAL_CACHE", "1")

import concourse.bass as bass
import concourse.tile as tile
from concourse import bacc, mybir
from concourse.bass_utils import run_bass_kernel_spmd


def _install_ntff_hook():
    """Provide antenv.axon_hooks (NTFF profiling registry) if the image
    lacks it, mirroring trn_agent_boot's ctypes hook. No-op on failure."""
    try:
        import antenv.axon_hooks  # noqa: F401
        return
    except ImportError:
        pass
    try:
        import contextlib
        import ctypes
        import types

        so_path = "/opt/axon/libaxon_pjrt.so"
        if not os.path.exists(so_path):
            return
        lib = ctypes.CDLL(so_path)
        if not hasattr(lib, "axon_start_nrt_profile"):
            return
        lib.axon_start_nrt_profile.argtypes = [
            ctypes.POINTER(ctypes.c_int64), ctypes.c_size_t]
        lib.axon_start_nrt_profile.restype = ctypes.c_int64
        lib.axon_stop_nrt_profile.argtypes = [ctypes.c_char_p]
        lib.axon_stop_nrt_profile.restype = ctypes.c_int64

        @contextlib.contextmanager
        def _hook(output_dir, device_ids):
            import jax
            jax.devices()
            if device_ids:
                ids = (ctypes.c_int64 * len(device_ids))(*device_ids)
                rc = lib.axon_start_nrt_profile(ids, len(device_ids))
            else:
                rc = lib.axon_start_nrt_profile(None, 0)
            if rc != 0:
                raise RuntimeError(f"axon_start_nrt_profile rc={rc}")
            try:
                yield
            finally:
                n = lib.axon_stop_nrt_profile(str(output_dir).encode())
                print(f"ntff profile: {n} file(s) -> {output_dir}",
                      file=sys.stderr)

        mod = types.ModuleType("antenv.axon_hooks")
        _state = {"hook": _hook}
        mod.set_axon_ntff_profile_hook = lambda h: _state.__setitem__("hook", h)
        mod.get_axon_ntff_profile_hook = lambda: _state["hook"]
        sys.modules["antenv.axon_hooks"] = mod
        import antenv
        antenv.axon_hooks = mod
    except Exception:
        pass


_install_ntff_hook()

F32 = mybir.dt.float32
BF16 = mybir.dt.bfloat16
FP8 = mybir.dt.float8e4
ACT_EXP = mybir.ActivationFunctionType.Exp
ACT_COPY = mybir.ActivationFunctionType.Copy
DR = mybir.MatmulPerfMode.DoubleRow

# Problem constants (hardcoded per contract).
HIDDEN = 1024
HEADS = 16
HEAD_DIM = 64
GAMMA = 0.5
B, S = 2, 2048
N_CORES = 8
GROUPS = N_CORES // B  # head groups per batch
HPC = HEADS // GROUPS  # heads per core
PAIRS = HPC // 2
LAG = 3                # kt software-pipeline lag between pt and ctx matmul
XH_PRESCALE = 64.0     # fp8 prescale; sim psum = PRESCALE^2 * gamma * sim


def emit_kernel(tc, aps, *, S_, C_, QB):
    nc = tc.nc
    CT = C_ // 128          # contraction tiles over hidden
    CC = CT // 2            # fp8 DoubleRow chunks (256 rows each)
    NKT = S_ // 128         # key tiles
    NQB = S_ // QB          # query blocks
    PB = 512                # projection free-block width
    NPB = S_ // PB
    D2 = HPC * HEAD_DIM     # per-core projected channels

    xT_d = aps["xT"]; scale_d = aps["scale"]
    wq_d = aps["wq"]; wk_d = aps["wk"]; wv_d = aps["wv"]; wo_d = aps["wo"]
    bq_d = aps["bq"]; bk_d = aps["bk"]; bv_d = aps["bv"]
    out_d = aps["out"]
    mask_d = aps.get("maskmul")

    from contextlib import ExitStack
    stack = ExitStack()

    # --- persistent SBUF tensors ---
    proj = stack.enter_context(tc.tile_pool(name="proj", bufs=1))
    qT_sb = proj.tile([128, PAIRS, S_], BF16)      # head pairs on 64-halves
    kT_sb = proj.tile([128, PAIRS, S_], BF16)
    v2_sb = proj.tile([128, NKT, HPC, HEAD_DIM + 1], BF16)
    xh8_sb = proj.tile([128, CC, 2, S_], FP8)      # DoubleRow interleaved
    wo_sb = proj.tile([128, PAIRS, C_], BF16)
    ctxT2_sb = proj.tile([128, PAIRS, S_], BF16)

    # ---------------- phase 1: load + projections ----------------
    with tc.tile_pool(name="xsp", bufs=1) as xsp, \
         tc.tile_pool(name="wsp", bufs=1) as wsp, \
         tc.tile_pool(name="prjps", bufs=2, space="PSUM") as prjps, \
         tc.tile_pool(name="vps", bufs=2, space="PSUM") as vps:

        # wk first (consumption order: kT computed first), then x, then rest
        w_sb = {}
        for c in range(CT):
            wt = wsp.tile([128, D2], BF16, tag=f"wk{c}", name=f"wk_{c}")
            nc.sync.dma_start(out=wt, in_=wk_d[c * 128:(c + 1) * 128, :])
            w_sb[("wk", c)] = wt
        xs = []
        for c in range(CT):
            xt = xsp.tile([128, S_], BF16, tag=f"xs{c}", name=f"xs_{c}")
            nc.sync.dma_start(out=xt, in_=xT_d[c * 128:(c + 1) * 128, :])
            xs.append(xt)
        for wname, w_d in (("wq", wq_d), ("wv", wv_d)):
            for c in range(CT):
                wt = wsp.tile([128, D2], BF16, tag=f"{wname}{c}",
                              name=f"{wname}_{c}")
                nc.sync.dma_start(out=wt, in_=w_d[c * 128:(c + 1) * 128, :])
                w_sb[(wname, c)] = wt
        nc.sync.dma_start(
            out=wo_sb, in_=wo_d.rearrange("(j p) o -> p j o", p=128))

        b_sb = {}
        for bname, b_d in (("bq", bq_d), ("bk", bk_d), ("bv", bv_d)):
            bt = wsp.tile([128, PAIRS, 1], F32, tag=f"b{bname}")
            nc.sync.dma_start(
                out=bt, in_=b_d.rearrange("(j p) one -> p j one", p=128))
            b_sb[bname] = bt

        # normalization scale row -> broadcast to all partitions (gpsimd)
        scr = wsp.tile([1, S_], BF16, tag="scr")
        nc.sync.dma_start(out=scr, in_=scale_d)
        scb = wsp.tile([128, S_], BF16, tag="scb")
        nc.gpsimd.partition_broadcast(scb, scr, channels=128)

        # bv broadcast row for the V bias add ([1, D2] varies along free dim)
        bvr = wsp.tile([1, D2], F32, tag="bvr")
        nc.sync.dma_start(out=bvr, in_=bv_d.rearrange("d one -> one d"))
        bvb = wsp.tile([128, D2], F32, tag="bvb")
        nc.gpsimd.partition_broadcast(bvb, bvr, channels=128)

        # q/k projections: W tiles stationary, x moving; accumulate over c
        for wname, bname, dest in (("wk", "bk", kT_sb), ("wq", "bq", qT_sb)):
            for nb in range(NPB):
                pss = [prjps.tile([128, PB], F32, tag=f"prj{j}",
                                  name=f"prj_{wname}_{nb}_{j}")
                       for j in range(PAIRS)]
                for c in range(CT):
                    for j in range(PAIRS):
                        nc.tensor.matmul(
                            pss[j],
                            w_sb[(wname, c)][:, j * 128:(j + 1) * 128],
                            xs[c][:, nb * PB:(nb + 1) * PB],
                            start=(c == 0),
                            stop=(c == CT - 1),
                        )
                for j in range(PAIRS):
                    nc.vector.tensor_scalar_add(
                        dest[:, j, nb * PB:(nb + 1) * PB], pss[j],
                        b_sb[bname][:, j, :])

        # V directly in [keys, dh] layout: x tile stationary, WvT moving
        for st in range(NKT):
            vp = vps.tile([128, D2], F32, tag="vp", name=f"vp_{st}")
            for c in range(CT):
                nc.tensor.matmul(
                    vp,
                    xs[c][:, st * 128:(st + 1) * 128],
                    w_sb[("wv", c)],
                    start=(c == 0),
                    stop=(c == CT - 1),
                )
            nc.vector.tensor_add(
                v2_sb[:, st, :, 0:HEAD_DIM],
                vp.rearrange("p (h d) -> p h d", h=HPC),
                bvb.rearrange("p (h d) -> p h d", h=HPC),
            )
        nc.vector.memset(v2_sb[:, :, :, HEAD_DIM:HEAD_DIM + 1], 1.0)

        # xh8: fp8 DoubleRow-interleaved normalized x (scaled columns)
        for c in range(CT):
            nc.vector.tensor_mul(
                xh8_sb[:, c // 2, c % 2, :], xs[c], scb)

    # ---------------- phase 2: attention main loop ----------------
    epool = stack.enter_context(tc.tile_pool(name="epool", bufs=2))
    espool = stack.enter_context(tc.tile_pool(name="espool", bufs=2))
    ptpool = stack.enter_context(tc.tile_pool(name="ptpool", bufs=4))
    smallpool = stack.enter_context(tc.tile_pool(name="smallpool", bufs=2))
    outstg = stack.enter_context(tc.tile_pool(name="outstg", bufs=3))
    mpool = (stack.enter_context(tc.tile_pool(name="mpool", bufs=2))
             if mask_d is not None else None)

    with tc.tile_pool(name="mmps", bufs=1, space="PSUM") as mmps, \
         tc.tile_pool(name="ctxps", bufs=1, space="PSUM") as ctxps:

        mmctr = [0]

        def mm_tile(name):
            t = mmps.tile([128, 2, QB], F32, tag=f"T{mmctr[0] % 3}",
                          name=name)
            mmctr[0] += 1
            return t

        def emit_ctx_pair(ctx_pair, j, kt, pt):
            for hi in range(2):
                nc.tensor.matmul(
                    ctx_pair[hi],
                    v2_sb[:, kt, 2 * j + hi, :],
                    pt[:, hi, :],
                    start=(kt == 0),
                    stop=(kt == NKT - 1),
                    skip_group_check=True,
                )

        def emit_out_qt(qt):
            # one query tile's partial out-projection: [128, 1024]
            op = mm_tile(f"op_{qt}")
            for ob in range(2):
                for j in range(PAIRS):
                    nc.tensor.matmul(
                        op[:, ob, :],
                        ctxT2_sb[:, j, qt * 128:(qt + 1) * 128],
                        wo_sb[:, j, ob * QB:(ob + 1) * QB],
                        start=(j == 0),
                        stop=(j == PAIRS - 1),
                    )
            ostg = outstg.tile([128, 2, QB], F32, tag="ostg",
                               name=f"ostg_{qt}")
            nc.vector.tensor_copy(ostg[:, 0, :], op[:, 0, :])
            nc.scalar.activation(out=ostg[:, 1, :], in_=op[:, 1, :],
                                 func=ACT_COPY)
            nc.sync.dma_start(
                out=out_d[qt * 128:(qt + 1) * 128, :],
                in_=ostg.rearrange("p a b -> p (a b)"),
            )

        def emit_divisions_pair(qb, j, ctx_pair):
            qsl = slice(qb * QB, (qb + 1) * QB)
            for hi in range(2):
                s0 = smallpool.tile([1, QB], F32, tag="s0",
                                    name=f"s0_{qb}_{j}_{hi}")
                nc.vector.tensor_copy(
                    s0, ctx_pair[hi][HEAD_DIM:HEAD_DIM + 1, :])
                r0 = smallpool.tile([1, QB], F32, tag="r0",
                                    name=f"r0_{qb}_{j}_{hi}")
                nc.vector.reciprocal_approx_fast(r0, s0)
                rb = smallpool.tile([HEAD_DIM, QB], F32, tag="rb",
                                    name=f"rb_{qb}_{j}_{hi}")
                nc.gpsimd.partition_broadcast(rb, r0, channels=HEAD_DIM)
                nc.vector.tensor_mul(
                    ctxT2_sb[hi * 64:hi * 64 + 64, j, qsl],
                    ctx_pair[hi][0:HEAD_DIM, :],
                    rb,
                )

        carry = None        # (qb, j, ctx_pair, undrained pops) of prev pass
        pending_out = None  # qb whose out-projection is ready to emit
        inv_ps2 = -1.0 / (XH_PRESCALE * XH_PRESCALE)
        for qb in range(NQB):
            qsl = slice(qb * QB, (qb + 1) * QB)
            e_tiles = {}
            for j in range(PAIRS):
                ctx_pair = [
                    ctxps.tile([HEAD_DIM + 1, QB], F32, tag=f"cx{hi}",
                               name=f"ctx_{qb}_{j}_{hi}")
                    for hi in range(2)
                ]
                pending = []
                for kt in range(NKT):
                    ksl = slice(kt * 128, (kt + 1) * 128)
                    if j == 0 and kt % 2 == 0:
                        # sim for kt, kt+1 (fp8 DoubleRow) and shared E
                        spt = mm_tile(f"sim_{qb}_{kt}")
                        for par in range(2):
                            kk = slice((kt + par) * 128, (kt + par + 1) * 128)
                            for cc in range(CC):
                                nc.tensor.matmul(
                                    spt[:, par, :],
                                    xh8_sb[:, cc, :, kk],
                                    xh8_sb[:, cc, :, qsl],
                                    start=(cc == 0),
                                    stop=(cc == CC - 1),
                                    perf_mode=DR,
                                )
                        e2 = epool.tile([128, 2, QB], BF16, tag=f"E{kt // 2}",
                                        name=f"E_{qb}_{kt}")
                        nc.scalar.activation(out=e2, in_=spt, func=ACT_EXP,
                                             scale=inv_ps2)
                        if mask_d is not None:
                            for par in range(2):
                                kk = slice((kt + par) * 128,
                                           (kt + par + 1) * 128)
                                m_sb = mpool.tile([128, QB], BF16, tag="msk")
                                nc.sync.dma_start(out=m_sb,
                                                  in_=mask_d[kk, qsl])
                                nc.vector.tensor_mul(
                                    e2[:, par, :], e2[:, par, :], m_sb)
                        e_tiles[kt // 2] = e2
                    eb = (e_tiles[kt // 2][:, kt % 2, :]
                          .unsqueeze(1).to_broadcast([128, 2, QB]))

                    sc_t = mm_tile(f"sc_{qb}_{kt}_{j}")
                    for hi in range(2):
                        pr = slice(hi * 64, hi * 64 + 64)
                        nc.tensor.matmul(
                            sc_t[:, hi, :],
                            kT_sb[pr, j, ksl],
                            qT_sb[pr, j, qsl],
                            start=True,
                            stop=True,
                        )
                    es_t = espool.tile([128, 2, QB], BF16, tag=f"es{j}",
                                       name=f"es_{qb}_{kt}_{j}")
                    nc.scalar.activation(out=es_t, in_=sc_t, func=ACT_EXP)
                    pt = ptpool.tile([128, 2, QB], BF16, tag=f"pt{j}",
                                     name=f"pt_{qb}_{kt}_{j}")
                    nc.vector.tensor_mul(pt, es_t, eb)
                    pending.append((kt, pt))

                    # drain the previous pass's pops, then its divisions
                    if carry is not None and kt <= 2:
                        cqb, cj, cctx, cpend = carry
                        k0, p0 = cpend.pop(0)
                        emit_ctx_pair(cctx, cj, k0, p0)
                        if not cpend:
                            emit_divisions_pair(cqb, cj, cctx)
                            if cj == PAIRS - 1:
                                pending_out = cqb
                            carry = None
                    if len(pending) > LAG:
                        k0, p0 = pending.pop(0)
                        emit_ctx_pair(ctx_pair, j, k0, p0)
                    # out-projection of the qb finished two passes ago
                    if j == 1 and 2 <= kt <= 5 and pending_out is not None:
                        emit_out_qt(pending_out * (QB // 128) + kt - 2)
                        if kt == 5:
                            pending_out = None
                carry = (qb, j, ctx_pair, pending)
        cqb, cj, cctx, cpend = carry
        for k0, p0 in cpend:
            emit_ctx_pair(cctx, cj, k0, p0)
        emit_divisions_pair(cqb, cj, cctx)
        for qt in range(cqb * (QB // 128), (cqb + 1) * (QB // 128)):
            emit_out_qt(qt)

    stack.close()


def build_nc(*, S_=S, C_=HIDDEN, QB=512, with_mask=False,
             enable_asserts=False):
    nc = bacc.Bacc(
        "TRN2", target_bir_lowering=False, debug=False,
        enable_asserts=enable_asserts,
    )
    D2 = HPC * HEAD_DIM
    aps = {}
    aps["xT"] = nc.dram_tensor("xT", [C_, S_], BF16, kind="ExternalInput").ap()
    aps["scale"] = nc.dram_tensor(
        "scale", [1, S_], BF16, kind="ExternalInput").ap()
    for n in ("wq", "wk", "wv"):
        aps[n] = nc.dram_tensor(n, [C_, D2], BF16, kind="ExternalInput").ap()
    aps["wo"] = nc.dram_tensor("wo", [D2, C_], BF16, kind="ExternalInput").ap()
    for n in ("bq", "bk", "bv"):
        aps[n] = nc.dram_tensor(n, [D2, 1], F32, kind="ExternalInput").ap()
    if with_mask:
        aps["maskmul"] = nc.dram_tensor(
            "maskmul", [S_, S_], BF16, kind="ExternalInput").ap()
    aps["out"] = nc.dram_tensor("out", [S_, C_], F32, kind="ExternalOutput").ap()

    with tile.TileContext(nc) as tc:
        emit_kernel(tc, aps, S_=S_, C_=C_, QB=QB)
    nc.compile()
    return nc


def host_prepare(x, attn_mask, Wq, bq, Wk, bk, Wv, bv, Wo, bo, *,
                 S_=S, C_=HIDDEN, n_cores=N_CORES):
    """Build the per-core input maps. Returns (in_maps, with_mask)."""
    import ml_dtypes
    BF = ml_dtypes.bfloat16
    x = np.asarray(x, np.float32)
    B_ = x.shape[0]
    groups = n_cores // B_
    Wq = np.asarray(Wq, np.float32); Wk = np.asarray(Wk, np.float32)
    Wv = np.asarray(Wv, np.float32); Wo = np.asarray(Wo, np.float32)
    bq = np.asarray(bq, np.float32); bk = np.asarray(bk, np.float32)
    bv = np.asarray(bv, np.float32)

    inv_sqrt_d = 1.0 / math.sqrt(HEAD_DIM)
    WqT = np.ascontiguousarray((Wq * inv_sqrt_d).T).astype(BF)
    WkT = np.ascontiguousarray(Wk.T).astype(BF)
    WvT = np.ascontiguousarray(Wv.T).astype(BF)
    WoT = np.ascontiguousarray(Wo.T).astype(BF)      # [C(c), C(o)]
    bq = bq * inv_sqrt_d

    mask = np.asarray(attn_mask)
    with_mask = bool(mask.any())
    maskmul = None
    if with_mask:
        # reference: where(mask, -inf) -> multiplicative 0/1 on exp values
        maskmul = np.where(mask.T, 0.0, 1.0).astype(BF)
        maskmul = np.ascontiguousarray(maskmul)

    in_maps = []
    for core in range(n_cores):
        b, g = divmod(core, groups)
        xb = x[b]                                   # [S, C]
        xT = np.ascontiguousarray(xb.T).astype(BF)  # [C, S]
        norms = np.linalg.norm(xb, axis=1)          # [S]
        scale = (XH_PRESCALE * math.sqrt(GAMMA)
                 / np.maximum(norms, 1e-12)).astype(BF)
        D2 = HPC * HEAD_DIM
        ch = slice(g * D2, (g + 1) * D2)
        m = {
            "xT": xT,
            "scale": scale.reshape(1, S_),
            "wq": np.ascontiguousarray(WqT[:, ch]),
            "wk": np.ascontiguousarray(WkT[:, ch]),
            "wv": np.ascontiguousarray(WvT[:, ch]),
            "wo": np.ascontiguousarray(WoT[ch, :]),
            "bq": np.ascontiguousarray(bq[ch]).reshape(-1, 1),
            "bk": np.ascontiguousarray(bk[ch]).reshape(-1, 1),
            "bv": np.ascontiguousarray(bv[ch]).reshape(-1, 1),
        }
        if with_mask:
            m["maskmul"] = maskmul
        in_maps.append(m)
    return in_maps, with_mask


_NC_CACHE = {}


def _get_nc(with_mask):
    key = with_mask
    if key not in _NC_CACHE:
        _NC_CACHE[key] = build_nc(with_mask=with_mask)
    return _NC_CACHE[key]


LAST_RESULTS = None


def kernel(**inputs):
    global LAST_RESULTS
    in_maps, with_mask = host_prepare(
        inputs["x"], inputs["attn_mask"],
        inputs["Wq"], inputs["bq"], inputs["Wk"], inputs["bk"],
        inputs["Wv"], inputs["bv"], inputs["Wo"], inputs["bo"],
    )
    nc = _get_nc(with_mask)
    res = run_bass_kernel_spmd(nc, in_maps, core_ids=list(range(N_CORES)))
    LAST_RESULTS = res
    bo = np.asarray(inputs["bo"], np.float32)
    out = np.zeros((B, S, HIDDEN), np.float32)
    groups = N_CORES // B
    for core in range(N_CORES):
        b = core // groups
        out[b] += np.asarray(res.results[core]["out"], np.float32)
    out += bo[None, None, :]
    return out


# revision 15
# speedup vs baseline: 1.8383x; 1.0138x over previous
"""DiversityAttention on 8 TRN2 NeuronCores (Bass/Tile), v3.

Sharding: data-parallel over batch (B=2) x tensor-parallel over heads
(16 heads -> 4 groups of 4). core = (b, g), b = core // 4, g = core % 4.
Each core computes full attention for its 4 heads over its batch and a
partial out-projection [S, HIDDEN]; the host sums the 4 partials per
batch and adds bo.

Keys-on-partitions orientation, all-bf16 matmuls except the fp8
DoubleRow similarity:
  qT = (Wq/8 @ x^T + bq/8)  [64h, S] bf16    (1/sqrt(dh) folded on host)
  kT = (Wk   @ x^T + bk)    [64h, S] bf16
  V  = x @ WvT + bv directly in [keys, dh] layout, bf16 + ones column
  xh8 = fp8e4(x^T * 64*sqrt(gamma)/||x||)  (DoubleRow-interleaved)
  per (qb, kt):
     sim_psum = xh8^T xh8 (fp8 DoubleRow) = 4096*gamma*sim
     E = exp(-sim_psum/4096) (ACT, scale folded)          [128, QB] bf16
     scores_psum = kT^T qT per head pair (row-packed 64-contraction)
     es = exp(scores_psum) (ACT reads PSUM)               bf16
     pt = es * E (DVE bf16 2x)
     ctx_psum[65, QB] += [V|1]^T pt  (per head, ones row = softmax sums)
  sim/scores/out-proj PSUM tiles rotate through two 2-bank tags so the
  exp of one tile overlaps matmuls into the other; ctx holds 4 banks.
  at qb end: copy sums -> recip_approx_fast -> gpsimd broadcast -> DVE
  mul gives ctxT2 bf16; qb's out-projection is spread over kt=1..4 of
  the next qb so the PE never idles at block boundaries.
"""

import math
import os
import sys

import numpy as np

for _p in ("/opt/trn_rl_repo",):
    if _p not in sys.path and os.path.isdir(_p):
        sys.path.insert(0, _p)

os.environ.setdefault("MYCRO_LOCAL_CACHE", "1")

import concourse.bass as bass
import concourse.tile as tile
from concourse import bacc, mybir
from concourse.bass_utils import run_bass_kernel_spmd


def _install_ntff_hook():
    """Provide antenv.axon_hooks (NTFF profiling registry) if the image
    lacks it, mirroring trn_agent_boot's ctypes hook. No-op on failure."""
    try:
        import antenv.axon_hooks  # noqa: F401
        return
    except ImportError:
        pass
    try:
        import contextlib
        import ctypes
        import types

        so_path = "/opt/axon/libaxon_pjrt.so"
        if not os.path.exists(so_path):
            return
        lib = ctypes.CDLL(so_path)
        if not hasattr(lib, "axon_start_nrt_profile"):
            return
        lib.axon_start_nrt_profile.argtypes = [
            ctypes.POINTER(ctypes.c_int64), ctypes.c_size_t]
        lib.axon_start_nrt_profile.restype = ctypes.c_int64
        lib.axon_stop_nrt_profile.argtypes = [ctypes.c_char_p]
        lib.axon_stop_nrt_profile.restype = ctypes.c_int64

        @contextlib.contextmanager
        def _hook(output_dir, device_ids):
            import jax
            jax.devices()
            if device_ids:
                ids = (ctypes.c_int64 * len(device_ids))(*device_ids)
                rc = lib.axon_start_nrt_profile(ids, len(device_ids))
            else:
                rc = lib.axon_start_nrt_profile(None, 0)
            if rc != 0:
                raise RuntimeError(f"axon_start_nrt_profile rc={rc}")
            try:
                yield
            finally:
                n = lib.axon_stop_nrt_profile(str(output_dir).encode())
                print(f"ntff profile: {n} file(s) -> {output_dir}",
                      file=sys.stderr)

        mod = types.ModuleType("antenv.axon_hooks")
        _state = {"hook": _hook}
        mod.set_axon_ntff_profile_hook = lambda h: _state.__setitem__("hook", h)
        mod.get_axon_ntff_profile_hook = lambda: _state["hook"]
        sys.modules["antenv.axon_hooks"] = mod
        import antenv
        antenv.axon_hooks = mod
    except Exception:
        pass


_install_ntff_hook()

F32 = mybir.dt.float32
BF16 = mybir.dt.bfloat16
FP8 = mybir.dt.float8e4
ACT_EXP = mybir.ActivationFunctionType.Exp
ACT_COPY = mybir.ActivationFunctionType.Copy
DR = mybir.MatmulPerfMode.DoubleRow

# Problem constants (hardcoded per contract).
HIDDEN = 1024
HEADS = 16
HEAD_DIM = 64
GAMMA = 0.5
B, S = 2, 2048
N_CORES = 8
GROUPS = N_CORES // B  # head groups per batch
HPC = HEADS // GROUPS  # heads per core
PAIRS = HPC // 2
LAG = 3                # kt software-pipeline lag between pt and ctx matmul
XH_PRESCALE = 64.0     # fp8 prescale; sim psum = PRESCALE^2 * gamma * sim


def emit_kernel(tc, aps, *, S_, C_, QB):
    nc = tc.nc
    CT = C_ // 128          # contraction tiles over hidden
    CC = CT // 2            # fp8 DoubleRow chunks (256 rows each)
    NKT = S_ // 128         # key tiles
    NQB = S_ // QB          # query blocks
    PB = 512                # projection free-block width
    NPB = S_ // PB
    D2 = HPC * HEAD_DIM     # per-core projected channels

    xT_d = aps["xT"]; scale_d = aps["scale"]
    wq_d = aps["wq"]; wk_d = aps["wk"]; wv_d = aps["wv"]; wo_d = aps["wo"]
    bq_d = aps["bq"]; bk_d = aps["bk"]; bv_d = aps["bv"]
    out_d = aps["out"]
    mask_d = aps.get("maskmul")

    from contextlib import ExitStack
    stack = ExitStack()

    # --- persistent SBUF tensors ---
    proj = stack.enter_context(tc.tile_pool(name="proj", bufs=1))
    qT_sb = proj.tile([128, PAIRS, S_], BF16)      # head pairs on 64-halves
    kT_sb = proj.tile([128, PAIRS, S_], BF16)
    v2_sb = proj.tile([128, NKT, HPC, HEAD_DIM + 1], BF16)
    xh8_sb = proj.tile([128, CC, 2, S_], FP8)      # DoubleRow interleaved
    wo_sb = proj.tile([128, PAIRS, C_], BF16)
    ctxT2_sb = proj.tile([128, PAIRS, S_], BF16)

    # ---------------- phase 1: load + projections ----------------
    with tc.tile_pool(name="xsp", bufs=1) as xsp, \
         tc.tile_pool(name="wsp", bufs=1) as wsp, \
         tc.tile_pool(name="prjps", bufs=2, space="PSUM") as prjps, \
         tc.tile_pool(name="vps", bufs=2, space="PSUM") as vps:

        # wk first (consumption order: kT computed first), then x, then rest
        w_sb = {}
        for c in range(CT):
            wt = wsp.tile([128, D2], BF16, tag=f"wk{c}", name=f"wk_{c}")
            nc.sync.dma_start(out=wt, in_=wk_d[c * 128:(c + 1) * 128, :])
            w_sb[("wk", c)] = wt
        xs = []
        H = S_ // 2
        for c in range(CT):
            xt = xsp.tile([128, S_], BF16, tag=f"xs{c}", name=f"xs_{c}")
            nc.sync.dma_start(out=xt[:, 0:H], in_=xT_d[c * 128:(c + 1) * 128, 0:H])
            xs.append(xt)
        for c in range(CT):
            nc.sync.dma_start(out=xs[c][:, H:S_],
                              in_=xT_d[c * 128:(c + 1) * 128, H:S_])
        for wname, w_d in (("wq", wq_d), ("wv", wv_d)):
            for c in range(CT):
                wt = wsp.tile([128, D2], BF16, tag=f"{wname}{c}",
                              name=f"{wname}_{c}")
                nc.sync.dma_start(out=wt, in_=w_d[c * 128:(c + 1) * 128, :])
                w_sb[(wname, c)] = wt
        nc.sync.dma_start(
            out=wo_sb, in_=wo_d.rearrange("(j p) o -> p j o", p=128))

        b_sb = {}
        for bname, b_d in (("bq", bq_d), ("bk", bk_d), ("bv", bv_d)):
            bt = wsp.tile([128, PAIRS, 1], F32, tag=f"b{bname}")
            nc.sync.dma_start(
                out=bt, in_=b_d.rearrange("(j p) one -> p j one", p=128))
            b_sb[bname] = bt

        # normalization scale row -> broadcast to all partitions (gpsimd)
        scr = wsp.tile([1, S_], BF16, tag="scr")
        nc.sync.dma_start(out=scr, in_=scale_d)
        scb = wsp.tile([128, S_], BF16, tag="scb")
        nc.gpsimd.partition_broadcast(scb, scr, channels=128)

        # bv broadcast row for the V bias add ([1, D2] varies along free dim)
        bvr = wsp.tile([1, D2], F32, tag="bvr")
        nc.sync.dma_start(out=bvr, in_=bv_d.rearrange("d one -> one d"))
        bvb = wsp.tile([128, D2], F32, tag="bvb")
        nc.gpsimd.partition_broadcast(bvb, bvr, channels=128)

        # q/k projections: W tiles stationary, x moving; accumulate over c
        def emit_proj_nb(wname, bname, dest, nb):
            pss = [prjps.tile([128, PB], F32, tag=f"prj{j}",
                              name=f"prj_{wname}_{nb}_{j}")
                   for j in range(PAIRS)]
            for c in range(CT):
                for j in range(PAIRS):
                    nc.tensor.matmul(
                        pss[j],
                        w_sb[(wname, c)][:, j * 128:(j + 1) * 128],
                        xs[c][:, nb * PB:(nb + 1) * PB],
                        start=(c == 0),
                        stop=(c == CT - 1),
                    )
            for j in range(PAIRS):
                nc.vector.tensor_scalar_add(
                    dest[:, j, nb * PB:(nb + 1) * PB], pss[j],
                    b_sb[bname][:, j, :])

        # V directly in [keys, dh] layout: x tile stationary, WvT moving
        def emit_v_st(st):
            vp = vps.tile([128, D2], F32, tag="vp", name=f"vp_{st}")
            for c in range(CT):
                nc.tensor.matmul(
                    vp,
                    xs[c][:, st * 128:(st + 1) * 128],
                    w_sb[("wv", c)],
                    start=(c == 0),
                    stop=(c == CT - 1),
                )
            nc.vector.tensor_add(
                v2_sb[:, st, :, 0:HEAD_DIM],
                vp.rearrange("p (h d) -> p h d", h=HPC),
                bvb.rearrange("p (h d) -> p h d", h=HPC),
            )

        # consume x half 0 first (its DMA lands first), then half 1
        emit_proj_nb("wk", "bk", kT_sb, 0)
        emit_proj_nb("wq", "bq", qT_sb, 0)
        for st in range(4):
            emit_v_st(st)
        emit_proj_nb("wk", "bk", kT_sb, 1)
        emit_proj_nb("wq", "bq", qT_sb, 1)
        for st in range(4, 8):
            emit_v_st(st)
        for nb in range(2, NPB):
            emit_proj_nb("wk", "bk", kT_sb, nb)
            emit_proj_nb("wq", "bq", qT_sb, nb)
        for st in range(8, NKT):
            emit_v_st(st)
        nc.vector.memset(v2_sb[:, :, :, HEAD_DIM:HEAD_DIM + 1], 1.0)

        # xh8: fp8 DoubleRow-interleaved normalized x (scaled columns)
        H2 = S_ // 2
        for c in range(CT):
            nc.vector.tensor_mul(
                xh8_sb[:, c // 2, c % 2, 0:H2], xs[c][:, 0:H2],
                scb[:, 0:H2])
        for c in range(CT):
            nc.vector.tensor_mul(
                xh8_sb[:, c // 2, c % 2, H2:S_], xs[c][:, H2:S_],
                scb[:, H2:S_])

    # ---------------- phase 2: attention main loop ----------------
    epool = stack.enter_context(tc.tile_pool(name="epool", bufs=2))
    espool = stack.enter_context(tc.tile_pool(name="espool", bufs=2))
    ptpool = stack.enter_context(tc.tile_pool(name="ptpool", bufs=4))
    smallpool = stack.enter_context(tc.tile_pool(name="smallpool", bufs=2))
    outstg = stack.enter_context(tc.tile_pool(name="outstg", bufs=3))
    mpool = (stack.enter_context(tc.tile_pool(name="mpool", bufs=2))
             if mask_d is not None else None)

    with tc.tile_pool(name="mmps", bufs=1, space="PSUM") as mmps, \
         tc.tile_pool(name="ctxps", bufs=1, space="PSUM") as ctxps:

        mmctr = [0]

        def mm_tile(name):
            t = mmps.tile([128, 2, QB], F32, tag=f"T{mmctr[0] % 3}",
                          name=name)
            mmctr[0] += 1
            return t

        def emit_ctx_pair(ctx_pair, j, kt, pt):
            for hi in range(2):
                nc.tensor.matmul(
                    ctx_pair[hi],
                    v2_sb[:, kt, 2 * j + hi, :],
                    pt[:, hi, :],
                    start=(kt == 0),
                    stop=(kt == NKT - 1),
                    skip_group_check=True,
                )

        def emit_out_qt(qt):
            # one query tile's partial out-projection: [128, 1024]
            op = mm_tile(f"op_{qt}")
            for ob in range(2):
                for j in range(PAIRS):
                    nc.tensor.matmul(
                        op[:, ob, :],
                        ctxT2_sb[:, j, qt * 128:(qt + 1) * 128],
                        wo_sb[:, j, ob * QB:(ob + 1) * QB],
                        start=(j == 0),
                        stop=(j == PAIRS - 1),
                    )
            ostg = outstg.tile([128, 2, QB], F32, tag="ostg",
                               name=f"ostg_{qt}")
            nc.vector.tensor_copy(ostg[:, 0, :], op[:, 0, :])
            nc.scalar.activation(out=ostg[:, 1, :], in_=op[:, 1, :],
                                 func=ACT_COPY)
            nc.sync.dma_start(
                out=out_d[qt * 128:(qt + 1) * 128, :],
                in_=ostg.rearrange("p a b -> p (a b)"),
            )

        def emit_divisions_pair(qb, j, ctx_pair):
            qsl = slice(qb * QB, (qb + 1) * QB)
            for hi in range(2):
                s0 = smallpool.tile([1, QB], F32, tag="s0",
                                    name=f"s0_{qb}_{j}_{hi}")
                nc.vector.tensor_copy(
                    s0, ctx_pair[hi][HEAD_DIM:HEAD_DIM + 1, :])
                r0 = smallpool.tile([1, QB], F32, tag="r0",
                                    name=f"r0_{qb}_{j}_{hi}")
                nc.vector.reciprocal_approx_fast(r0, s0)
                rb = smallpool.tile([HEAD_DIM, QB], F32, tag="rb",
                                    name=f"rb_{qb}_{j}_{hi}")
                nc.gpsimd.partition_broadcast(rb, r0, channels=HEAD_DIM)
                nc.vector.tensor_mul(
                    ctxT2_sb[hi * 64:hi * 64 + 64, j, qsl],
                    ctx_pair[hi][0:HEAD_DIM, :],
                    rb,
                )

        carry = None        # (qb, j, ctx_pair, undrained pops) of prev pass
        pending_out = None  # qb whose out-projection is ready to emit
        inv_ps2 = -1.0 / (XH_PRESCALE * XH_PRESCALE)
        for qb in range(NQB):
            qsl = slice(qb * QB, (qb + 1) * QB)
            e_tiles = {}
            for j in range(PAIRS):
                ctx_pair = [
                    ctxps.tile([HEAD_DIM + 1, QB], F32, tag=f"cx{hi}",
                               name=f"ctx_{qb}_{j}_{hi}")
                    for hi in range(2)
                ]
                pending = []
                for kt in range(NKT):
                    ksl = slice(kt * 128, (kt + 1) * 128)
                    if j == 0 and kt % 2 == 0:
                        # sim for kt, kt+1 (fp8 DoubleRow) and shared E
                        spt = mm_tile(f"sim_{qb}_{kt}")
                        for par in range(2):
                            kk = slice((kt + par) * 128, (kt + par + 1) * 128)
                            for cc in range(CC):
                                nc.tensor.matmul(
                                    spt[:, par, :],
                                    xh8_sb[:, cc, :, kk],
                                    xh8_sb[:, cc, :, qsl],
                                    start=(cc == 0),
                                    stop=(cc == CC - 1),
                                    perf_mode=DR,
                                )
                        e2 = epool.tile([128, 2, QB], BF16, tag=f"E{kt // 2}",
                                        name=f"E_{qb}_{kt}")
                        nc.scalar.activation(out=e2, in_=spt, func=ACT_EXP,
                                             scale=inv_ps2)
                        if mask_d is not None:
                            for par in range(2):
                                kk = slice((kt + par) * 128,
                                           (kt + par + 1) * 128)
                                m_sb = mpool.tile([128, QB], BF16, tag="msk")
                                nc.sync.dma_start(out=m_sb,
                                                  in_=mask_d[kk, qsl])
                                nc.vector.tensor_mul(
                                    e2[:, par, :], e2[:, par, :], m_sb)
                        e_tiles[kt // 2] = e2
                    eb = (e_tiles[kt // 2][:, kt % 2, :]
                          .unsqueeze(1).to_broadcast([128, 2, QB]))

                    sc_t = mm_tile(f"sc_{qb}_{kt}_{j}")
                    for hi in range(2):
                        pr = slice(hi * 64, hi * 64 + 64)
                        nc.tensor.matmul(
                            sc_t[:, hi, :],
                            kT_sb[pr, j, ksl],
                            qT_sb[pr, j, qsl],
                            start=True,
                            stop=True,
                        )
                    es_t = espool.tile([128, 2, QB], BF16, tag=f"es{j}",
                                       name=f"es_{qb}_{kt}_{j}")
                    nc.scalar.activation(out=es_t, in_=sc_t, func=ACT_EXP)
                    pt = ptpool.tile([128, 2, QB], BF16, tag=f"pt{j}",
                                     name=f"pt_{qb}_{kt}_{j}")
                    nc.vector.tensor_mul(pt, es_t, eb)
                    pending.append((kt, pt))

                    # drain the previous pass's pops, then its divisions
                    if carry is not None and kt <= 2:
                        cqb, cj, cctx, cpend = carry
                        k0, p0 = cpend.pop(0)
                        emit_ctx_pair(cctx, cj, k0, p0)
                        if not cpend:
                            emit_divisions_pair(cqb, cj, cctx)
                            if cj == PAIRS - 1:
                                pending_out = cqb
                            carry = None
                    if len(pending) > LAG:
                        k0, p0 = pending.pop(0)
                        emit_ctx_pair(ctx_pair, j, k0, p0)
                    # out-projection of the qb finished two passes ago
                    if j == 1 and 2 <= kt <= 5 and pending_out is not None:
                        emit_out_qt(pending_out * (QB // 128) + kt - 2)
                        if kt == 5:
                            pending_out = None
                carry = (qb, j, ctx_pair, pending)
        cqb, cj, cctx, cpend = carry
        for k0, p0 in cpend:
            emit_ctx_pair(cctx, cj, k0, p0)
        emit_divisions_pair(cqb, cj, cctx)
        for qt in range(cqb * (QB // 128), (cqb + 1) * (QB // 128)):
            emit_out_qt(qt)

    stack.close()


def build_nc(*, S_=S, C_=HIDDEN, QB=512, with_mask=False,
             enable_asserts=False):
    nc = bacc.Bacc(
        "TRN2", target_bir_lowering=False, debug=False,
        enable_asserts=enable_asserts,
    )
    D2 = HPC * HEAD_DIM
    aps = {}
    aps["xT"] = nc.dram_tensor("xT", [C_, S_], BF16, kind="ExternalInput").ap()
    aps["scale"] = nc.dram_tensor(
        "scale", [1, S_], BF16, kind="ExternalInput").ap()
    for n in ("wq", "wk", "wv"):
        aps[n] = nc.dram_tensor(n, [C_, D2], BF16, kind="ExternalInput").ap()
    aps["wo"] = nc.dram_tensor("wo", [D2, C_], BF16, kind="ExternalInput").ap()
    for n in ("bq", "bk", "bv"):
        aps[n] = nc.dram_tensor(n, [D2, 1], F32, kind="ExternalInput").ap()
    if with_mask:
        aps["maskmul"] = nc.dram_tensor(
            "maskmul", [S_, S_], BF16, kind="ExternalInput").ap()
    aps["out"] = nc.dram_tensor("out", [S_, C_], F32, kind="ExternalOutput").ap()

    with tile.TileContext(nc) as tc:
        emit_kernel(tc, aps, S_=S_, C_=C_, QB=QB)
    nc.compile()
    return nc


def host_prepare(x, attn_mask, Wq, bq, Wk, bk, Wv, bv, Wo, bo, *,
                 S_=S, C_=HIDDEN, n_cores=N_CORES):
    """Build the per-core input maps. Returns (in_maps, with_mask)."""
    import ml_dtypes
    BF = ml_dtypes.bfloat16
    x = np.asarray(x, np.float32)
    B_ = x.shape[0]
    groups = n_cores // B_
    Wq = np.asarray(Wq, np.float32); Wk = np.asarray(Wk, np.float32)
    Wv = np.asarray(Wv, np.float32); Wo = np.asarray(Wo, np.float32)
    bq = np.asarray(bq, np.float32); bk = np.asarray(bk, np.float32)
    bv = np.asarray(bv, np.float32)

    inv_sqrt_d = 1.0 / math.sqrt(HEAD_DIM)
    WqT = np.ascontiguousarray((Wq * inv_sqrt_d).T).astype(BF)
    WkT = np.ascontiguousarray(Wk.T).astype(BF)
    WvT = np.ascontiguousarray(Wv.T).astype(BF)
    WoT = np.ascontiguousarray(Wo.T).astype(BF)      # [C(c), C(o)]
    bq = bq * inv_sqrt_d

    mask = np.asarray(attn_mask)
    with_mask = bool(mask.any())
    maskmul = None
    if with_mask:
        # reference: where(mask, -inf) -> multiplicative 0/1 on exp values
        maskmul = np.where(mask.T, 0.0, 1.0).astype(BF)
        maskmul = np.ascontiguousarray(maskmul)

    in_maps = []
    for core in range(n_cores):
        b, g = divmod(core, groups)
        xb = x[b]                                   # [S, C]
        xT = np.ascontiguousarray(xb.T).astype(BF)  # [C, S]
        norms = np.linalg.norm(xb, axis=1)          # [S]
        scale = (XH_PRESCALE * math.sqrt(GAMMA)
                 / np.maximum(norms, 1e-12)).astype(BF)
        D2 = HPC * HEAD_DIM
        ch = slice(g * D2, (g + 1) * D2)
        m = {
            "xT": xT,
            "scale": scale.reshape(1, S_),
            "wq": np.ascontiguousarray(WqT[:, ch]),
            "wk": np.ascontiguousarray(WkT[:, ch]),
            "wv": np.ascontiguousarray(WvT[:, ch]),
            "wo": np.ascontiguousarray(WoT[ch, :]),
            "bq": np.ascontiguousarray(bq[ch]).reshape(-1, 1),
            "bk": np.ascontiguousarray(bk[ch]).reshape(-1, 1),
            "bv": np.ascontiguousarray(bv[ch]).reshape(-1, 1),
        }
        if with_mask:
            m["maskmul"] = maskmul
        in_maps.append(m)
    return in_maps, with_mask


_NC_CACHE = {}


def _get_nc(with_mask):
    key = with_mask
    if key not in _NC_CACHE:
        _NC_CACHE[key] = build_nc(with_mask=with_mask)
    return _NC_CACHE[key]


LAST_RESULTS = None


def kernel(**inputs):
    global LAST_RESULTS
    in_maps, with_mask = host_prepare(
        inputs["x"], inputs["attn_mask"],
        inputs["Wq"], inputs["bq"], inputs["Wk"], inputs["bk"],
        inputs["Wv"], inputs["bv"], inputs["Wo"], inputs["bo"],
    )
    nc = _get_nc(with_mask)
    res = run_bass_kernel_spmd(nc, in_maps, core_ids=list(range(N_CORES)))
    LAST_RESULTS = res
    bo = np.asarray(inputs["bo"], np.float32)
    out = np.zeros((B, S, HIDDEN), np.float32)
    groups = N_CORES // B
    for core in range(N_CORES):
        b = core // groups
        out[b] += np.asarray(res.results[core]["out"], np.float32)
    out += bo[None, None, :]
    return out
